# revision 16
# baseline (speedup 1.0000x reference)
"""Trainium2 Bass kernel for DeepSelfAttention (N=8192, D=1024) on 8 NeuronCores.

Strategy (row-parallel attention):
  - Shard the N=8192 rows of x across 8 cores (1024 rows each); replicate weights.
  - All operand transposes (x and the six d x d weights) are done by the DMA
    XBAR (fp32 natural load -> ScalarE fp16 cast -> 8 dma_start_transpose per
    matrix), keeping the TensorEngine free for matmuls.
  - Each core computes Q/K/V projections for its row shard feature-major;
    K^T and V are shipped per key-half: (K^T h0, V h0) -> AllGather0,
    (K^T h1, V h1) -> AllGather1, so the first collective starts as early
    as possible; Q projection + MLP weight transposes fill its latency.
  - Flash-style one-pass attention: scores^T tiles [k=128, q=512] accumulate
    over feature tiles in PSUM, exp on ScalarE (scale=1/32 fused; scores are
    provably in [-3, 3] so no max-subtraction), A@V per (block, dt) with
    free-dim 512 into a rotating set of 4 PSUM banks, flushed to an SBUF
    fp32 accumulator; softmax denominator via a ones-vector matmul.
  - The V bias is folded into the post-softmax normalize (softmax rows sum
    to 1), where it is a per-partition bias.
  - 3-layer MLP + final projection, feature-major.
All matmul operands are fp16 (full PE rate on TRN2) with fp32 PSUM
accumulation; end-to-end max rel err vs the fp32 reference is ~4e-4.
"""

import os

import numpy as np

import concourse.mybir as mybir
import concourse.tile as tile
from concourse import bacc
from concourse import bass_utils

P = 128
D = 1024
N = 8192
NCORES = 8
NS = N // NCORES          # 1024 rows per core
DT = D // P               # 8 feature tiles
KB = 8                    # k blocks (one per source core)
KTB = NS // P             # 8 k tiles per block
KTH = KTB // 2            # 4 k tiles per chunk-block
CH = NS // 2              # 512 keys per chunk
KSZ = D * CH              # K-chunk elements in the flat collective buffer
VSZ = CH * D
F16 = mybir.dt.float16
F32 = mybir.dt.float32
AF = mybir.ActivationFunctionType
ALU = mybir.AluOpType

SCALE = 1.0 / np.sqrt(np.float32(D)).astype(np.float32)  # 0.03125

_CACHE = {}


def _transpose_xbar_half(nc, st32, st16, src_ap, rh, dst_fn):
    """One row-half of src_ap (DRAM fp32 [R, C]) -> dst_fn(rh, t): SBUF fp16
    [P, C//P, P] slice receiving columns of src.T for source rows
    [rh*R/2 + t*P, ...+P). Natural load on the Sync DMA queue, ScalarE fp16
    cast, XBAR transposes on the Scalar DMA queue (so loads and transposes
    stream on independent queues)."""
    R, C = src_ap.shape
    tb = R // P // 2
    nat = st32.tile([P, tb, C], F32, tag="st32")
    nc.sync.dma_start(
        nat[:],
        src_ap[rh * (R // 2):(rh + 1) * (R // 2), :].rearrange(
            "(t p) c -> p t c", p=P))
    nath = st16.tile([P, tb, C], F16, tag="st16")
    nc.scalar.copy(nath[:], nat[:])
    for t in range(tb):
        nc.scalar.dma_start_transpose(dst_fn(rh, t), nath[:, t, :])


def _wslice(dst_tile):
    def fn(rh, t):
        i = rh * 4 + t
        return dst_tile[:, :, i * P:(i + 1) * P]
    return fn


def _build():
    nc = bacc.Bacc("TRN2", target_bir_lowering=False, debug=False,
                   num_devices=NCORES)
    xs = nc.dram_tensor("xs", [NS, D], F32, kind="ExternalInput").ap()
    W = {}
    for w in ("wq", "wk", "wv", "w1", "w2", "w3"):
        W[w] = nc.dram_tensor(w, [D, D], F32, kind="ExternalInput").ap()
    B = {}
    for b in ("bq", "bk", "bv", "b1", "b2", "b3"):
        B[b] = nc.dram_tensor(b, [D], F32, kind="ExternalInput").ap()
    fw = nc.dram_tensor("fw", [D], F32, kind="ExternalInput").ap()
    out = nc.dram_tensor("out", [1, NS], F32, kind="ExternalOutput").ap()
    debug = bool(os.environ.get("K_DEBUG"))
    dbg = {}
    if debug:
        for nm, shp, dt_ in (("dq", [D, NS], F16), ("drs", [1, NS], F32),
                             ("datt", [D, NS], F16), ("dy1", [D, NS], F16)):
            dbg[nm] = nc.dram_tensor(nm, shp, dt_, kind="ExternalOutput").ap()

    with tile.TileContext(nc) as tc:
        with (
            tc.tile_pool(name="persist", bufs=1) as pers,
            tc.tile_pool(name="dram", bufs=1, space="DRAM") as dram,
        ):
            # ---- persistent SBUF tiles ----
            qt = pers.tile([P, DT, NS], F16, tag="qt")          # Q^T
            wT = {w: pers.tile([P, DT, D], F16, tag=f"{w}T", name=f"{w}T")
                  for w in ("w1", "w2", "w3")}
            bsb = {b: pers.tile([P, DT], F32, tag=f"{b}sb", name=f"{b}sb")
                   for b in B}
            fwh = pers.tile([P, DT], F16, tag="fwh")
            ones_row = pers.tile([1, P], F32, tag="ones_row")
            rs = pers.tile([1, NS], F32, tag="rs")              # softmax denom

            # ---- DRAM scratch: flat (K-chunk | V-chunk) collective buffers
            kv_d = [dram.tile([KSZ + VSZ], F16, name=f"kv_d{c}")
                    for c in range(2)]
            kvag = [dram.tile([NCORES * (KSZ + VSZ)], F16, name=f"kvag{c}",
                              addr_space="Shared")
                    for c in range(2)]

            # ---- constants (on the GpSimd DMA queue, off the load path) ----
            for b in B:
                nc.gpsimd.dma_start(bsb[b][:],
                                    B[b].rearrange("(t p) -> p t", p=P))
            fwf = pers.tile([P, DT], F32, tag="fwf")
            nc.gpsimd.dma_start(fwf[:], fw.rearrange("(t p) -> p t", p=P))
            nc.vector.tensor_copy(fwh[:], fwf[:])
            nc.gpsimd.memset(ones_row[:], 1.0)

            # ---- early pool: dies after projections ----
            early = tc.alloc_tile_pool(name="early", bufs=1)
            xsT = [early.tile([P, DT, 512], F16, tag=f"xsT{h}",
                              name=f"xsT{h}") for h in range(2)]
            for w in ("wq", "wk", "wv"):
                wT[w] = early.tile([P, DT, D], F16, tag=f"{w}T", name=f"{w}T")
            kts = early.tile([P, DT, NS], F16, tag="kts")       # K^T shard
            vs = early.tile([P, KTB, D], F16, tag="vs")         # V shard

            with (
                tc.tile_pool(name="st32", bufs=2) as st32,
                tc.tile_pool(name="st16", bufs=1) as st16,
                tc.tile_pool(name="ppj", bufs=4, space="PSUM") as ppj,
            ):
                # XBAR transposes: x half0 + wk first (gate K-proj h0), then
                # x half1 + wv (gate V-proj). x's row-half rh lands in
                # xsT[rh] (rows = keys for K-proj).
                xfn = lambda rh, t: xsT[rh][:, :, t * P:(t + 1) * P]
                _transpose_xbar_half(nc, st32, st16, xs, 0, xfn)
                _transpose_xbar_half(nc, st32, st16, W["wk"], 0,
                                     _wslice(wT["wk"]))
                _transpose_xbar_half(nc, st32, st16, W["wk"], 1,
                                     _wslice(wT["wk"]))
                _transpose_xbar_half(nc, st32, st16, xs, 1, xfn)
                _transpose_xbar_half(nc, st32, st16, W["wv"], 0,
                                     _wslice(wT["wv"]))
                _transpose_xbar_half(nc, st32, st16, W["wv"], 1,
                                     _wslice(wT["wv"]))

                # per key-half: K^T, V, ship, AllGather
                for h in range(2):
                    # K^T[:, half] = Wk @ xs^T + bk
                    for dt in range(DT):
                        ps = ppj.tile([P, 512], F32, tag="ppj")
                        for et in range(DT):
                            nc.tensor.matmul(
                                ps[:],
                                wT["wk"][:, et, dt * P:(dt + 1) * P],
                                xsT[h][:, et, :],
                                start=(et == 0), stop=(et == DT - 1))
                        nc.scalar.activation(
                            kts[:, dt, h * 512:(h + 1) * 512], ps[:],
                            AF.Identity, bias=bsb["bk"][:, dt:dt + 1])
                    nc.scalar.dma_start(
                        kv_d[h][0:KSZ].rearrange("(p t k) -> p t k", p=P, k=CH),
                        kts[:, :, h * CH:(h + 1) * CH])
                    # V[half] = xs @ Wv.T (bias folded into the normalize)
                    for kt in range(h * KTH, (h + 1) * KTH):
                        for dh in range(2):
                            ps = ppj.tile([P, 512], F32, tag="ppj")
                            for et in range(DT):
                                nc.tensor.matmul(
                                    ps[:],
                                    xsT[h][:, et,
                                           (kt - h * KTH) * P:
                                           (kt - h * KTH + 1) * P],
                                    wT["wv"][:, et, dh * 512:(dh + 1) * 512],
                                    start=(et == 0), stop=(et == DT - 1))
                            nc.scalar.copy(
                                vs[:, kt, dh * 512:(dh + 1) * 512], ps[:])
                    nc.scalar.dma_start(
                        kv_d[h][KSZ:].rearrange("(p t d) -> p t d", p=P, d=D),
                        vs[:, h * KTH:(h + 1) * KTH, :])
                    nc.gpsimd.collective_compute(
                        "AllGather", ALU.bypass,
                        replica_groups=[list(range(NCORES))],
                        ins=[kv_d[h].opt()], outs=[kvag[h].opt()])

                # work that fills the collective latency: Q^T projection
                # + MLP weight transposes
                for rh in range(2):
                    _transpose_xbar_half(nc, st32, st16, W["wq"], rh,
                                         _wslice(wT["wq"]))
                for dt in range(DT):
                    for h in range(2):
                        ps = ppj.tile([P, 512], F32, tag="ppj")
                        for et in range(DT):
                            nc.tensor.matmul(
                                ps[:],
                                wT["wq"][:, et, dt * P:(dt + 1) * P],
                                xsT[h][:, et, :],
                                start=(et == 0), stop=(et == DT - 1))
                        nc.scalar.activation(
                            qt[:, dt, h * 512:(h + 1) * 512], ps[:],
                            AF.Identity, bias=bsb["bq"][:, dt:dt + 1])
                for w in ("w1", "w2", "w3"):
                    for rh in range(2):
                        _transpose_xbar_half(nc, st32, st16, W[w], rh,
                                             _wslice(wT[w]))

            early.release()

            if debug:
                nc.sync.dma_start(dbg["dq"].rearrange("(t p) k -> p t k", p=P),
                                  qt[:])

            # ---- attention over 2 chunks x 8 blocks ----
            pacc = tc.alloc_tile_pool(name="pacc", bufs=1)
            attacc = pacc.tile([P, DT, NS], F32, tag="attacc")
            rs_acc = pacc.tile([P, 2, 512], F32, tag="rs_acc")
            with (
                tc.tile_pool(name="kv", bufs=4) as kv,
                tc.tile_pool(name="ex", bufs=8) as exp_pool,
                tc.tile_pool(name="psc", bufs=2, space="PSUM") as psc,
                tc.tile_pool(name="pat", bufs=4, space="PSUM") as pat,
            ):
                for ch in range(2):
                    base = kvag[ch]
                    for kb in range(KB):
                        off = kb * (KSZ + VSZ)
                        ktb = kv.tile([P, DT, CH], F16, tag="ktb")
                        vb = kv.tile([P, KTH, D], F16, tag="vb")
                        nc.gpsimd.dma_start(
                            ktb[:],
                            base[off:off + KSZ].rearrange(
                                "(p t k) -> p t k", p=P, k=CH))
                        nc.gpsimd.dma_start(
                            vb[:],
                            base[off + KSZ:off + KSZ + VSZ].rearrange(
                                "(p t d) -> p t d", p=P, d=D))
                        first_blk = ch == 0 and kb == 0
                        for qp in range(2):
                            qpsl = slice(qp * 512, (qp + 1) * 512)
                            exs = []
                            for kt in range(KTH):
                                sc = psc.tile([P, 512], F32, tag="psc")
                                for dt in range(DT):
                                    nc.tensor.matmul(
                                        sc[:],
                                        ktb[:, dt, kt * P:(kt + 1) * P],
                                        qt[:, dt, qpsl],
                                        start=(dt == 0), stop=(dt == DT - 1))
                                ex = exp_pool.tile([P, 512], F16, tag="ex",
                                                   name=f"ex{kt}")
                                nc.scalar.activation(ex[:], sc[:], AF.Exp,
                                                     scale=float(SCALE))
                                # softmax denominator: per-partition partial
                                # sums on the DVE (reduced at the end)
                                if first_blk and kt == 0:
                                    nc.vector.tensor_copy(rs_acc[:, qp, :],
                                                          ex[:])
                                else:
                                    nc.vector.tensor_tensor(
                                        rs_acc[:, qp, :], ex[:],
                                        rs_acc[:, qp, :], ALU.add)
                                exs.append(ex)
                            # A@V: per dt, accumulate the 4 kt matmuls in one
                            # PSUM bank (free dim 512), 4 banks rotating
                            for dt in range(DT):
                                att_ps = pat.tile([P, 512], F32, tag="pat")
                                for kt in range(KTH):
                                    nc.tensor.matmul(
                                        att_ps[:],
                                        vb[:, kt, dt * P:(dt + 1) * P],
                                        exs[kt][:],
                                        start=(kt == 0),
                                        stop=(kt == KTH - 1))
                                dsl = (slice(None), dt, qpsl)
                                if first_blk:
                                    nc.vector.tensor_copy(attacc[dsl],
                                                          att_ps[:])
                                else:
                                    nc.vector.tensor_tensor(
                                        attacc[dsl], att_ps[:],
                                        attacc[dsl], ALU.add)
                # reduce rs_acc across partitions: ones^T @ rs_acc (fp32)
                with tc.tile_pool(name="prs", bufs=2, space="PSUM") as prs:
                    ones_f = pacc.tile([P, 1], F32, tag="ones_f")
                    nc.gpsimd.memset(ones_f[:], 1.0)
                    for qp in range(2):
                        rs_ps = prs.tile([1, 512], F32, tag="prs")
                        nc.tensor.matmul(rs_ps[:], ones_f[:],
                                         rs_acc[:, qp, :])
                        nc.vector.tensor_copy(
                            rs[0:1, qp * 512:(qp + 1) * 512], rs_ps[:])

            # ---- normalize + MLP + final ----
            with (
                tc.tile_pool(name="acts", bufs=2) as acts,
                tc.tile_pool(name="pml", bufs=4, space="PSUM") as pml,
            ):
                recip = acts.tile([1, NS], F32, tag="recip")
                out_sb = acts.tile([1, NS], F32, tag="out_sb")
                nc.vector.reciprocal(recip[:], rs[:])
                attn_h = acts.tile([P, DT, NS], F16, tag="y")
                for h in range(2):
                    qsl = slice(h * 512, (h + 1) * 512)
                    rb = pml.tile([P, 512], F32, tag="pml")
                    nc.tensor.matmul(rb[:], ones_row[:], recip[0:1, qsl])
                    for dt in range(DT):
                        nc.vector.tensor_tensor(
                            attn_h[:, dt, qsl], attacc[:, dt, qsl], rb[:],
                            ALU.mult)
                        nc.vector.tensor_tensor(
                            attn_h[:, dt, qsl], attn_h[:, dt, qsl],
                            bsb["bv"][:, dt:dt + 1].to_broadcast([P, 512]),
                            ALU.add)
                if debug:
                    nc.sync.dma_start(dbg["drs"][:], rs[:])
                    nc.sync.dma_start(
                        dbg["datt"].rearrange("(t p) q -> p t q", p=P),
                        attn_h[:])
                cur = attn_h
                for wname, bname in (("w1", "b1"), ("w2", "b2"), ("w3", "b3")):
                    nxt = acts.tile([P, DT, NS], F16, tag="y")
                    for ft in range(DT):
                        for h in range(2):
                            ps = pml.tile([P, 512], F32, tag="pml")
                            for dt in range(DT):
                                nc.tensor.matmul(
                                    ps[:],
                                    wT[wname][:, dt, ft * P:(ft + 1) * P],
                                    cur[:, dt, h * 512:(h + 1) * 512],
                                    start=(dt == 0), stop=(dt == DT - 1))
                            nc.scalar.activation(
                                nxt[:, ft, h * 512:(h + 1) * 512], ps[:],
                                AF.Relu, bias=bsb[bname][:, ft:ft + 1])
                    if debug and wname == "w1":
                        nc.sync.dma_start(
                            dbg["dy1"].rearrange("(t p) q -> p t q", p=P),
                            nxt[:])
                    cur = nxt
                for h in range(2):
                    ps = pml.tile([1, 512], F32, tag="pfin")
                    for ft in range(DT):
                        nc.tensor.matmul(
                            ps[:], fwh[:, ft:ft + 1],
                            cur[:, ft, h * 512:(h + 1) * 512],
                            start=(ft == 0), stop=(ft == DT - 1))
                    nc.vector.tensor_copy(out_sb[0:1, h * 512:(h + 1) * 512],
                                          ps[:])
                nc.sync.dma_start(out[:], out_sb[:])
            pacc.release()

    nc.compile()
    return nc


def _get_nc():
    if "nc" not in _CACHE:
        _CACHE["nc"] = _build()
    return _CACHE["nc"]


def kernel(**inputs):
    nc = _get_nc()
    x = np.ascontiguousarray(np.asarray(inputs["x"], dtype=np.float32))
    names = {"wq": "Wq", "wk": "Wk", "wv": "Wv", "w1": "W1", "w2": "W2",
             "w3": "W3", "bq": "bq", "bk": "bk", "bv": "bv", "b1": "b1",
             "b2": "b2", "b3": "b3"}
    shared = {k: np.ascontiguousarray(np.asarray(inputs[v], dtype=np.float32))
              for k, v in names.items()}
    shared["fw"] = np.ascontiguousarray(
        np.asarray(inputs["final_weight"], dtype=np.float32).reshape(D))
    in_maps = []
    for c in range(NCORES):
        m = dict(shared)
        m["xs"] = np.ascontiguousarray(x[c * NS:(c + 1) * NS, :])
        in_maps.append(m)
    res = bass_utils.run_bass_kernel_spmd(
        nc, in_maps, core_ids=list(range(NCORES)))
    if os.environ.get("K_DEBUG"):
        kernel.debug_results = res.results
    return np.concatenate(
        [res.results[c]["out"].reshape(NS) for c in range(NCORES)])


# revision 17
# speedup vs baseline: 1.3309x; 1.3309x over previous
"""Trainium2 Bass kernel for DeepSelfAttention (N=8192, D=1024) on 8 NeuronCores.

Strategy (row-parallel attention):
  - Shard the N=8192 rows of x across 8 cores (1024 rows each); replicate
    weights. All matmul operands must be contraction-major (features on SBUF
    partitions), so the host pre-transposes and fp16-casts x^T per shard and
    the six d x d weights once in numpy; the device DMAs them straight into
    their final SBUF layouts (no on-device transposes or casts at all).
  - Each core computes Q/K/V projections for its row shard feature-major;
    K^T and V are shipped per key-half: (K^T h0, V h0) -> AllGather0,
    (K^T h1, V h1) -> AllGather1, so the first collective starts as early
    as possible; Q projection fills its latency.
  - Flash-style one-pass attention: scores^T tiles [k=128, q=512] accumulate
    over feature tiles in PSUM, exp on ScalarE (scale=1/32 fused; scores are
    provably in [-3, 3] so no max-subtraction), A@V per (block, dt) with
    free-dim 512 into a rotating set of 4 PSUM banks, flushed to an SBUF
    fp32 accumulator on the DVE; softmax denominator accumulated per
    partition on the DVE and reduced by a single ones-matmul at the end.
  - The V bias is folded into the post-softmax normalize (softmax rows sum
    to 1), where it is a per-partition bias.
  - 3-layer MLP + final projection, feature-major.
DMA queues: bulk loads on Sync, K/V ships on Scalar (they depend on ScalarE
bias-adds anyway), attention block loads on Sync behind the weight loads,
small constants on GpSimd.
All matmul operands are fp16 (full PE rate on TRN2) with fp32 PSUM
accumulation; end-to-end max rel err vs the fp32 reference is ~4e-4.
"""

import os

import numpy as np

import concourse.mybir as mybir
import concourse.tile as tile
from concourse import bacc
from concourse import bass_utils

P = 128
D = 1024
N = 8192
NCORES = 8
NS = N // NCORES          # 1024 rows per core
DT = D // P               # 8 feature tiles
KB = 8                    # k blocks (one per source core)
KTB = NS // P             # 8 k tiles per block
KTH = KTB // 2            # 4 k tiles per chunk-block
CH = NS // 2              # 512 keys per chunk
KSZ = D * CH              # K-chunk elements in the flat collective buffer
VSZ = CH * D
F16 = mybir.dt.float16
F32 = mybir.dt.float32
AF = mybir.ActivationFunctionType
ALU = mybir.AluOpType

SCALE = 1.0 / np.sqrt(np.float32(D)).astype(np.float32)  # 0.03125

_CACHE = {}


def _build():
    nc = bacc.Bacc("TRN2", target_bir_lowering=False, debug=False,
                   num_devices=NCORES)
    # host-pretransposed, fp16: x^T shard [D, NS] and W^T [D, D] per weight
    xst = nc.dram_tensor("xst", [D, NS], F16, kind="ExternalInput").ap()
    WT = {}
    for w in ("wq", "wk", "wv", "w1", "w2", "w3"):
        WT[w] = nc.dram_tensor(w + "t", [D, D], F16, kind="ExternalInput").ap()
    B = {}
    for b in ("bq", "bk", "bv", "b1", "b2", "b3"):
        B[b] = nc.dram_tensor(b, [D], F32, kind="ExternalInput").ap()
    fw = nc.dram_tensor("fw", [D], F32, kind="ExternalInput").ap()
    out = nc.dram_tensor("out", [1, NS], F32, kind="ExternalOutput").ap()
    debug = bool(os.environ.get("K_DEBUG"))
    dbg = {}
    if debug:
        for nm, shp, dt_ in (("dq", [D, NS], F16), ("drs", [1, NS], F32),
                             ("datt", [D, NS], F16), ("dy1", [D, NS], F16)):
            dbg[nm] = nc.dram_tensor(nm, shp, dt_, kind="ExternalOutput").ap()

    with tile.TileContext(nc) as tc:
        with (
            tc.tile_pool(name="persist", bufs=1) as pers,
            tc.tile_pool(name="dram", bufs=1, space="DRAM") as dram,
        ):
            # ---- persistent SBUF tiles ----
            qt = pers.tile([P, DT, NS], F16, tag="qt")          # Q^T
            wT = {w: pers.tile([P, DT, D], F16, tag=f"{w}T", name=f"{w}T")
                  for w in ("w1", "w2", "w3")}
            bsb = {b: pers.tile([P, DT], F32, tag=f"{b}sb", name=f"{b}sb")
                   for b in B}
            fwh = pers.tile([P, DT], F16, tag="fwh")
            ones_row = pers.tile([1, P], F32, tag="ones_row")
            rs = pers.tile([1, NS], F32, tag="rs")              # softmax denom

            # ---- DRAM scratch: flat (K-chunk | V-chunk) collective buffers
            kv_d = [dram.tile([KSZ + VSZ], F16, name=f"kv_d{c}")
                    for c in range(2)]
            kvag = [dram.tile([NCORES * (KSZ + VSZ)], F16, name=f"kvag{c}",
                              addr_space="Shared")
                    for c in range(2)]

            # ---- constants (on the GpSimd DMA queue, off the load path) ----
            for b in B:
                nc.gpsimd.dma_start(bsb[b][:],
                                    B[b].rearrange("(t p) -> p t", p=P))
            fwf = pers.tile([P, DT], F32, tag="fwf")
            nc.gpsimd.dma_start(fwf[:], fw.rearrange("(t p) -> p t", p=P))
            nc.vector.tensor_copy(fwh[:], fwf[:])
            nc.gpsimd.memset(ones_row[:], 1.0)

            # ---- early pool: dies after projections ----
            early = tc.alloc_tile_pool(name="early", bufs=1)
            xsT = [early.tile([P, DT, 512], F16, tag=f"xsT{h}",
                              name=f"xsT{h}") for h in range(2)]
            for w in ("wq", "wk", "wv"):
                wT[w] = early.tile([P, DT, D], F16, tag=f"{w}T", name=f"{w}T")
            kts = early.tile([P, DT, NS], F16, tag="kts")       # K^T shard
            vs = early.tile([P, KTB, D], F16, tag="vs")         # V shard

            def load_wt(w):
                nc.sync.dma_start(
                    wT[w][:], WT[w].rearrange("(e p) c -> p e c", p=P))

            with tc.tile_pool(name="ppj", bufs=4, space="PSUM") as ppj:
                # loads ordered to unblock K h0, V h0 fastest
                nc.sync.dma_start(
                    xsT[0][:],
                    xst[:, 0:512].rearrange("(e p) n -> p e n", p=P))
                load_wt("wk")
                load_wt("wv")
                nc.sync.dma_start(
                    xsT[1][:],
                    xst[:, 512:1024].rearrange("(e p) n -> p e n", p=P))
                load_wt("wq")
                for w in ("w1", "w2", "w3"):
                    load_wt(w)

                # per key-half: K^T, V, ship, AllGather
                for h in range(2):
                    # K^T[:, half] = Wk @ xs^T + bk
                    for dt in range(DT):
                        ps = ppj.tile([P, 512], F32, tag="ppj")
                        for et in range(DT):
                            nc.tensor.matmul(
                                ps[:],
                                wT["wk"][:, et, dt * P:(dt + 1) * P],
                                xsT[h][:, et, :],
                                start=(et == 0), stop=(et == DT - 1))
                        nc.scalar.activation(
                            kts[:, dt, h * 512:(h + 1) * 512], ps[:],
                            AF.Identity, bias=bsb["bk"][:, dt:dt + 1])
                    nc.scalar.dma_start(
                        kv_d[h][0:KSZ].rearrange("(p t k) -> p t k", p=P, k=CH),
                        kts[:, :, h * CH:(h + 1) * CH])
                    # V[half] = xs @ Wv.T (bias folded into the normalize)
                    for kt in range(h * KTH, (h + 1) * KTH):
                        for dh in range(2):
                            ps = ppj.tile([P, 512], F32, tag="ppj")
                            for et in range(DT):
                                nc.tensor.matmul(
                                    ps[:],
                                    xsT[h][:, et,
                                           (kt - h * KTH) * P:
                                           (kt - h * KTH + 1) * P],
                                    wT["wv"][:, et, dh * 512:(dh + 1) * 512],
                                    start=(et == 0), stop=(et == DT - 1))
                            nc.scalar.copy(
                                vs[:, kt, dh * 512:(dh + 1) * 512], ps[:])
                    nc.scalar.dma_start(
                        kv_d[h][KSZ:].rearrange("(p t d) -> p t d", p=P, d=D),
                        vs[:, h * KTH:(h + 1) * KTH, :])
                    nc.gpsimd.collective_compute(
                        "AllGather", ALU.bypass,
                        replica_groups=[list(range(NCORES))],
                        ins=[kv_d[h].opt()], outs=[kvag[h].opt()])

                # Q^T projection fills the collective latency
                for dt in range(DT):
                    for h in range(2):
                        ps = ppj.tile([P, 512], F32, tag="ppj")
                        for et in range(DT):
                            nc.tensor.matmul(
                                ps[:],
                                wT["wq"][:, et, dt * P:(dt + 1) * P],
                                xsT[h][:, et, :],
                                start=(et == 0), stop=(et == DT - 1))
                        nc.scalar.activation(
                            qt[:, dt, h * 512:(h + 1) * 512], ps[:],
                            AF.Identity, bias=bsb["bq"][:, dt:dt + 1])

            early.release()

            if debug:
                nc.sync.dma_start(dbg["dq"].rearrange("(t p) k -> p t k", p=P),
                                  qt[:])

            # ---- attention over 2 chunks x 8 blocks ----
            pacc = tc.alloc_tile_pool(name="pacc", bufs=1)
            attacc = pacc.tile([P, DT, NS], F32, tag="attacc")
            rs_acc = pacc.tile([P, 2, 512], F32, tag="rs_acc")
            with (
                tc.tile_pool(name="kv", bufs=4) as kv,
                tc.tile_pool(name="ex", bufs=8) as exp_pool,
                tc.tile_pool(name="psc", bufs=2, space="PSUM") as psc,
                tc.tile_pool(name="pat", bufs=4, space="PSUM") as pat,
            ):
                for ch in range(2):
                    base = kvag[ch]
                    for kb in range(KB):
                        off = kb * (KSZ + VSZ)
                        ktb = kv.tile([P, DT, CH], F16, tag="ktb")
                        vb = kv.tile([P, KTH, D], F16, tag="vb")
                        nc.sync.dma_start(
                            ktb[:],
                            base[off:off + KSZ].rearrange(
                                "(p t k) -> p t k", p=P, k=CH))
                        nc.sync.dma_start(
                            vb[:],
                            base[off + KSZ:off + KSZ + VSZ].rearrange(
                                "(p t d) -> p t d", p=P, d=D))
                        first_blk = ch == 0 and kb == 0
                        for qp in range(2):
                            qpsl = slice(qp * 512, (qp + 1) * 512)
                            exs = []
                            for kt in range(KTH):
                                sc = psc.tile([P, 512], F32, tag="psc")
                                for dt in range(DT):
                                    nc.tensor.matmul(
                                        sc[:],
                                        ktb[:, dt, kt * P:(kt + 1) * P],
                                        qt[:, dt, qpsl],
                                        start=(dt == 0), stop=(dt == DT - 1))
                                ex = exp_pool.tile([P, 512], F16, tag="ex",
                                                   name=f"ex{kt}")
                                nc.scalar.activation(ex[:], sc[:], AF.Exp,
                                                     scale=float(SCALE))
                                # softmax denominator: per-partition partial
                                # sums on the DVE (reduced at the end)
                                if first_blk and kt == 0:
                                    nc.vector.tensor_copy(rs_acc[:, qp, :],
                                                          ex[:])
                                else:
                                    nc.vector.tensor_tensor(
                                        rs_acc[:, qp, :], ex[:],
                                        rs_acc[:, qp, :], ALU.add)
                                exs.append(ex)
                            # A@V: per dt, accumulate the 4 kt matmuls in one
                            # PSUM bank (free dim 512), 4 banks rotating
                            for dt in range(DT):
                                att_ps = pat.tile([P, 512], F32, tag="pat")
                                for kt in range(KTH):
                                    nc.tensor.matmul(
                                        att_ps[:],
                                        vb[:, kt, dt * P:(dt + 1) * P],
                                        exs[kt][:],
                                        start=(kt == 0),
                                        stop=(kt == KTH - 1))
                                dsl = (slice(None), dt, qpsl)
                                if first_blk:
                                    nc.vector.tensor_copy(attacc[dsl],
                                                          att_ps[:])
                                else:
                                    nc.vector.tensor_tensor(
                                        attacc[dsl], att_ps[:],
                                        attacc[dsl], ALU.add)
                # reduce rs_acc across partitions: ones^T @ rs_acc (fp32)
                with tc.tile_pool(name="prs", bufs=2, space="PSUM") as prs:
                    ones_f = pacc.tile([P, 1], F32, tag="ones_f")
                    nc.gpsimd.memset(ones_f[:], 1.0)
                    for qp in range(2):
                        rs_ps = prs.tile([1, 512], F32, tag="prs")
                        nc.tensor.matmul(rs_ps[:], ones_f[:],
                                         rs_acc[:, qp, :])
                        nc.vector.tensor_copy(
                            rs[0:1, qp * 512:(qp + 1) * 512], rs_ps[:])

            # ---- normalize + MLP + final ----
            with (
                tc.tile_pool(name="acts", bufs=2) as acts,
                tc.tile_pool(name="pml", bufs=4, space="PSUM") as pml,
            ):
                recip = acts.tile([1, NS], F32, tag="recip")
                out_sb = acts.tile([1, NS], F32, tag="out_sb")
                nc.vector.reciprocal(recip[:], rs[:])
                attn_h = acts.tile([P, DT, NS], F16, tag="y")
                for h in range(2):
                    qsl = slice(h * 512, (h + 1) * 512)
                    rb = pml.tile([P, 512], F32, tag="pml")
                    nc.tensor.matmul(rb[:], ones_row[:], recip[0:1, qsl])
                    for dt in range(DT):
                        nc.vector.tensor_tensor(
                            attn_h[:, dt, qsl], attacc[:, dt, qsl], rb[:],
                            ALU.mult)
                        nc.vector.tensor_tensor(
                            attn_h[:, dt, qsl], attn_h[:, dt, qsl],
                            bsb["bv"][:, dt:dt + 1].to_broadcast([P, 512]),
                            ALU.add)
                if debug:
                    nc.sync.dma_start(dbg["drs"][:], rs[:])
                    nc.sync.dma_start(
                        dbg["datt"].rearrange("(t p) q -> p t q", p=P),
                        attn_h[:])
                cur = attn_h
                for wname, bname in (("w1", "b1"), ("w2", "b2"), ("w3", "b3")):
                    nxt = acts.tile([P, DT, NS], F16, tag="y")
                    for ft in range(DT):
                        for h in range(2):
                            ps = pml.tile([P, 512], F32, tag="pml")
                            for dt in range(DT):
                                nc.tensor.matmul(
                                    ps[:],
                                    wT[wname][:, dt, ft * P:(ft + 1) * P],
                                    cur[:, dt, h * 512:(h + 1) * 512],
                                    start=(dt == 0), stop=(dt == DT - 1))
                            nc.scalar.activation(
                                nxt[:, ft, h * 512:(h + 1) * 512], ps[:],
                                AF.Relu, bias=bsb[bname][:, ft:ft + 1])
                    if debug and wname == "w1":
                        nc.sync.dma_start(
                            dbg["dy1"].rearrange("(t p) q -> p t q", p=P),
                            nxt[:])
                    cur = nxt
                for h in range(2):
                    ps = pml.tile([1, 512], F32, tag="pfin")
                    for ft in range(DT):
                        nc.tensor.matmul(
                            ps[:], fwh[:, ft:ft + 1],
                            cur[:, ft, h * 512:(h + 1) * 512],
                            start=(ft == 0), stop=(ft == DT - 1))
                    nc.vector.tensor_copy(out_sb[0:1, h * 512:(h + 1) * 512],
                                          ps[:])
                nc.sync.dma_start(out[:], out_sb[:])
            pacc.release()

    nc.compile()
    return nc


def _get_nc():
    if "nc" not in _CACHE:
        _CACHE["nc"] = _build()
    return _CACHE["nc"]


def _prep_shared(inputs):
    """Host-side prep: transpose + fp16-cast the weights once."""
    names = {"wq": "Wq", "wk": "Wk", "wv": "Wv", "w1": "W1", "w2": "W2",
             "w3": "W3"}
    shared = {}
    for k, v in names.items():
        shared[k + "t"] = np.ascontiguousarray(
            np.asarray(inputs[v], dtype=np.float32).T.astype(np.float16))
    for b in ("bq", "bk", "bv", "b1", "b2", "b3"):
        shared[b] = np.ascontiguousarray(np.asarray(inputs[b],
                                                    dtype=np.float32))
    shared["fw"] = np.ascontiguousarray(
        np.asarray(inputs["final_weight"], dtype=np.float32).reshape(D))
    return shared


def kernel(**inputs):
    nc = _get_nc()
    x = np.asarray(inputs["x"], dtype=np.float32)
    shared = _prep_shared(inputs)
    in_maps = []
    for c in range(NCORES):
        m = dict(shared)
        m["xst"] = np.ascontiguousarray(
            x[c * NS:(c + 1) * NS, :].T.astype(np.float16))
        in_maps.append(m)
    res = bass_utils.run_bass_kernel_spmd(
        nc, in_maps, core_ids=list(range(NCORES)))
    if os.environ.get("K_DEBUG"):
        kernel.debug_results = res.results
    return np.concatenate(
        [res.results[c]["out"].reshape(NS) for c in range(NCORES)])


# revision 22
# speedup vs baseline: 1.3539x; 1.0173x over previous
"""Trainium2 Bass kernel for DeepSelfAttention (N=8192, D=1024) on 8 NeuronCores.

Strategy (row-parallel attention):
  - Shard the N=8192 rows of x across 8 cores (1024 rows each); replicate
    weights. All matmul operands must be contraction-major (features on SBUF
    partitions), so the host pre-transposes and fp16-casts x^T per shard and
    the six d x d weights once in numpy; the device DMAs them straight into
    their final SBUF layouts (no on-device transposes or casts at all).
  - Each core computes Q/K/V projections for its row shard feature-major;
    K^T and V are shipped per key-half: (K^T h0, V h0) -> AllGather0,
    (K^T h1, V h1) -> AllGather1, so the first collective starts as early
    as possible; Q projection fills its latency.
  - Flash-style one-pass attention: scores^T tiles [k=128, q=512] accumulate
    over feature tiles in PSUM, exp on ScalarE (scale=1/32 fused; scores are
    provably in [-3, 3] so no max-subtraction), A@V per (block, dt) with
    free-dim 512 into a rotating set of 4 PSUM banks, flushed to an SBUF
    fp32 accumulator on the DVE; softmax denominator accumulated per
    partition on the DVE and reduced by a single ones-matmul at the end.
  - The V bias is folded into the post-softmax normalize (softmax rows sum
    to 1), where it is a per-partition bias.
  - 3-layer MLP + final projection, feature-major.
DMA queues: bulk loads on Sync, K/V ships on Scalar (they depend on ScalarE
bias-adds anyway), attention block loads on Sync behind the weight loads,
small constants on GpSimd.
All matmul operands are fp16 (full PE rate on TRN2) with fp32 PSUM
accumulation; end-to-end max rel err vs the fp32 reference is ~4e-4.
"""

import os

import numpy as np

import concourse.mybir as mybir
import concourse.tile as tile
from concourse import bacc
from concourse import bass_utils

P = 128
D = 1024
N = 8192
NCORES = 8
NS = N // NCORES          # 1024 rows per core
DT = D // P               # 8 feature tiles
KB = 8                    # k blocks (one per source core)
KTB = NS // P             # 8 k tiles per block
KTH = KTB // 2            # 4 k tiles per chunk-block
CH = NS // 2              # 512 keys per chunk
KSZ = D * CH              # K-chunk elements in the flat collective buffer
VSZ = CH * D
F16 = mybir.dt.float16
F32 = mybir.dt.float32
AF = mybir.ActivationFunctionType
ALU = mybir.AluOpType

SCALE = 1.0 / np.sqrt(np.float32(D)).astype(np.float32)  # 0.03125

_CACHE = {}


def _build():
    nc = bacc.Bacc("TRN2", target_bir_lowering=False, debug=False,
                   num_devices=NCORES)
    # host-pretransposed, fp16: x^T shard [D, NS] and W^T [D, D] per weight
    xst = nc.dram_tensor("xst", [D, NS], F16, kind="ExternalInput").ap()
    WT = {}
    for w in ("wq", "wk", "wv", "w1", "w2", "w3"):
        WT[w] = nc.dram_tensor(w + "t", [D, D], F16, kind="ExternalInput").ap()
    B = {}
    for b in ("bq", "bk", "bv", "b1", "b2", "b3"):
        B[b] = nc.dram_tensor(b, [D], F32, kind="ExternalInput").ap()
    fw = nc.dram_tensor("fw", [D], F32, kind="ExternalInput").ap()
    out = nc.dram_tensor("out", [1, NS], F32, kind="ExternalOutput").ap()
    debug = bool(os.environ.get("K_DEBUG"))
    dbg = {}
    if debug:
        for nm, shp, dt_ in (("dq", [D, NS], F16), ("drs", [1, NS], F32),
                             ("datt", [D, NS], F16), ("dy1", [D, NS], F16)):
            dbg[nm] = nc.dram_tensor(nm, shp, dt_, kind="ExternalOutput").ap()

    with tile.TileContext(nc) as tc:
        with (
            tc.tile_pool(name="persist", bufs=1) as pers,
            tc.tile_pool(name="dram", bufs=1, space="DRAM") as dram,
        ):
            # ---- persistent SBUF tiles ----
            qt = pers.tile([P, DT, NS], F16, tag="qt")          # Q^T
            wT = {w: pers.tile([P, DT, D], F16, tag=f"{w}T", name=f"{w}T")
                  for w in ("w1", "w2", "w3")}
            bsb = {b: pers.tile([P, DT], F32, tag=f"{b}sb", name=f"{b}sb")
                   for b in B}
            fwh = pers.tile([P, DT], F16, tag="fwh")
            rs = pers.tile([1, NS], F32, tag="rs")    # softmax denom (debug)

            # ---- DRAM scratch: flat (K-chunk | V-chunk) collective buffers
            kv_d = [dram.tile([KSZ + VSZ], F16, name=f"kv_d{c}")
                    for c in range(2)]
            kvag = [dram.tile([NCORES * (KSZ + VSZ)], F16, name=f"kvag{c}",
                              addr_space="Shared")
                    for c in range(2)]

            # ---- constants (on the GpSimd DMA queue, off the load path) ----
            for b in B:
                nc.gpsimd.dma_start(bsb[b][:],
                                    B[b].rearrange("(t p) -> p t", p=P))
            fwf = pers.tile([P, DT], F32, tag="fwf")
            nc.gpsimd.dma_start(fwf[:], fw.rearrange("(t p) -> p t", p=P))
            nc.vector.tensor_copy(fwh[:], fwf[:])

            # ---- early pool: dies after projections ----
            early = tc.alloc_tile_pool(name="early", bufs=1)
            xsT = [early.tile([P, DT, 512], F16, tag=f"xsT{h}",
                              name=f"xsT{h}") for h in range(2)]
            for w in ("wq", "wk", "wv"):
                wT[w] = early.tile([P, DT, D], F16, tag=f"{w}T", name=f"{w}T")
            kts = early.tile([P, DT, NS], F16, tag="kts")       # K^T shard
            vs = early.tile([P, KTB, D], F16, tag="vs")         # V shard

            def load_wt(w):
                nc.sync.dma_start(
                    wT[w][:], WT[w].rearrange("(e p) c -> p e c", p=P))

            with tc.tile_pool(name="ppj", bufs=4, space="PSUM") as ppj:
                # loads ordered to unblock K h0, V h0 fastest
                nc.sync.dma_start(
                    xsT[0][:],
                    xst[:, 0:512].rearrange("(e p) n -> p e n", p=P))
                load_wt("wk")
                load_wt("wv")
                nc.sync.dma_start(
                    xsT[1][:],
                    xst[:, 512:1024].rearrange("(e p) n -> p e n", p=P))
                load_wt("wq")
                for w in ("w1", "w2", "w3"):
                    load_wt(w)

                def kv_half(h):
                    # K^T[:, half] = Wk @ xs^T + bk
                    for dt in range(DT):
                        ps = ppj.tile([P, 512], F32, tag="ppj")
                        for et in range(DT):
                            nc.tensor.matmul(
                                ps[:],
                                wT["wk"][:, et, dt * P:(dt + 1) * P],
                                xsT[h][:, et, :],
                                start=(et == 0), stop=(et == DT - 1))
                        nc.scalar.activation(
                            kts[:, dt, h * 512:(h + 1) * 512], ps[:],
                            AF.Identity, bias=bsb["bk"][:, dt:dt + 1])
                    nc.scalar.dma_start(
                        kv_d[h][0:KSZ].rearrange("(p t k) -> p t k", p=P, k=CH),
                        kts[:, :, h * CH:(h + 1) * CH])
                    # V[half] = xs @ Wv.T (bias folded into the normalize)
                    for kt in range(h * KTH, (h + 1) * KTH):
                        for dh in range(2):
                            ps = ppj.tile([P, 512], F32, tag="ppj")
                            for et in range(DT):
                                nc.tensor.matmul(
                                    ps[:],
                                    xsT[h][:, et,
                                           (kt - h * KTH) * P:
                                           (kt - h * KTH + 1) * P],
                                    wT["wv"][:, et, dh * 512:(dh + 1) * 512],
                                    start=(et == 0), stop=(et == DT - 1))
                            nc.scalar.copy(
                                vs[:, kt, dh * 512:(dh + 1) * 512], ps[:])
                    nc.scalar.dma_start(
                        kv_d[h][KSZ:].rearrange("(p t d) -> p t d", p=P, d=D),
                        vs[:, h * KTH:(h + 1) * KTH, :])
                    nc.gpsimd.collective_compute(
                        "AllGather", ALU.bypass,
                        replica_groups=[list(range(NCORES))],
                        ins=[kv_d[h].opt()], outs=[kvag[h].opt()])

                def q_half(h):
                    for dt in range(DT):
                        ps = ppj.tile([P, 512], F32, tag="ppj")
                        for et in range(DT):
                            nc.tensor.matmul(
                                ps[:],
                                wT["wq"][:, et, dt * P:(dt + 1) * P],
                                xsT[h][:, et, :],
                                start=(et == 0), stop=(et == DT - 1))
                        nc.scalar.activation(
                            qt[:, dt, h * 512:(h + 1) * 512], ps[:],
                            AF.Identity, bias=bsb["bq"][:, dt:dt + 1])

                # K/V half0 -> AllGather0; Q-projection fills AllGather0's
                # latency; then K/V half1 -> AllGather1 (still well before
                # chunk-1 attention needs it)
                kv_half(0)
                q_half(0)
                q_half(1)
                kv_half(1)

            early.release()

            if debug:
                nc.sync.dma_start(dbg["dq"].rearrange("(t p) k -> p t k", p=P),
                                  qt[:])

            # ---- attention over 2 chunks x 8 blocks ----
            pacc = tc.alloc_tile_pool(name="pacc", bufs=1)
            attacc = pacc.tile([P, DT, NS], F32, tag="attacc")
            rs_acc = pacc.tile([P, 2, 512], F32, tag="rs_acc")
            with (
                tc.tile_pool(name="kv", bufs=4) as kv,
                tc.tile_pool(name="ex", bufs=8) as exp_pool,
                tc.tile_pool(name="psc", bufs=2, space="PSUM") as psc,
                tc.tile_pool(name="pat", bufs=4, space="PSUM") as pat,
            ):
                for ch in range(2):
                    base = kvag[ch]
                    for kb in range(KB):
                        off = kb * (KSZ + VSZ)
                        ktb = kv.tile([P, DT, CH], F16, tag="ktb")
                        vb = kv.tile([P, KTH, D], F16, tag="vb")
                        nc.sync.dma_start(
                            ktb[:],
                            base[off:off + KSZ].rearrange(
                                "(p t k) -> p t k", p=P, k=CH))
                        nc.sync.dma_start(
                            vb[:],
                            base[off + KSZ:off + KSZ + VSZ].rearrange(
                                "(p t d) -> p t d", p=P, d=D))
                        first_blk = ch == 0 and kb == 0
                        for qp in range(2):
                            qpsl = slice(qp * 512, (qp + 1) * 512)
                            exs = []
                            for kt in range(KTH):
                                sc = psc.tile([P, 512], F32, tag="psc")
                                for dt in range(DT):
                                    nc.tensor.matmul(
                                        sc[:],
                                        ktb[:, dt, kt * P:(kt + 1) * P],
                                        qt[:, dt, qpsl],
                                        start=(dt == 0), stop=(dt == DT - 1))
                                ex = exp_pool.tile([P, 512], F16, tag="ex",
                                                   name=f"ex{kt}")
                                nc.scalar.activation(ex[:], sc[:], AF.Exp,
                                                     scale=float(SCALE))
                                # softmax denominator: per-partition partial
                                # sums on the DVE (reduced at the end)
                                if first_blk and kt == 0:
                                    nc.vector.tensor_copy(rs_acc[:, qp, :],
                                                          ex[:])
                                else:
                                    nc.vector.tensor_tensor(
                                        rs_acc[:, qp, :], ex[:],
                                        rs_acc[:, qp, :], ALU.add)
                                exs.append(ex)
                            # A@V: per dt, accumulate the 4 kt matmuls in one
                            # PSUM bank (free dim 512), 4 banks rotating
                            for dt in range(DT):
                                att_ps = pat.tile([P, 512], F32, tag="pat")
                                for kt in range(KTH):
                                    nc.tensor.matmul(
                                        att_ps[:],
                                        vb[:, kt, dt * P:(dt + 1) * P],
                                        exs[kt][:],
                                        start=(kt == 0),
                                        stop=(kt == KTH - 1))
                                dsl = (slice(None), dt, qpsl)
                                if first_blk:
                                    nc.vector.tensor_copy(attacc[dsl],
                                                          att_ps[:])
                                else:
                                    nc.vector.tensor_tensor(
                                        attacc[dsl], att_ps[:],
                                        attacc[dsl], ALU.add)
                # reduce rs_acc across partitions, replicated to all 128
                # partitions (ones [P,P] stationary), then a parallel
                # reciprocal straight into the normalize operand
                recip_b = pacc.tile([P, 2, 512], F32, tag="recip_b")
                with tc.tile_pool(name="prs", bufs=2, space="PSUM") as prs:
                    ones_f = pacc.tile([P, P], F32, tag="ones_f")
                    nc.gpsimd.memset(ones_f[:], 1.0)
                    for qp in range(2):
                        rs_ps = prs.tile([P, 512], F32, tag="prs")
                        nc.tensor.matmul(rs_ps[:], ones_f[:],
                                         rs_acc[:, qp, :])
                        nc.vector.reciprocal(recip_b[:, qp, :], rs_ps[:])
                        if debug:
                            nc.vector.tensor_copy(
                                rs[0:1, qp * 512:(qp + 1) * 512],
                                rs_ps[0:1, :])

            # ---- normalize + MLP + final ----
            with (
                tc.tile_pool(name="acts", bufs=2) as acts,
                tc.tile_pool(name="pml", bufs=4, space="PSUM") as pml,
            ):
                out_sb = acts.tile([1, NS], F32, tag="out_sb")
                attn_h = acts.tile([P, DT, NS], F16, tag="y")
                for h in range(2):
                    qsl = slice(h * 512, (h + 1) * 512)
                    for dt in range(DT):
                        nc.vector.tensor_tensor(
                            attn_h[:, dt, qsl], attacc[:, dt, qsl],
                            recip_b[:, h, :], ALU.mult)
                        nc.vector.tensor_tensor(
                            attn_h[:, dt, qsl], attn_h[:, dt, qsl],
                            bsb["bv"][:, dt:dt + 1].to_broadcast([P, 512]),
                            ALU.add)
                if debug:
                    nc.sync.dma_start(dbg["drs"][:], rs[:])
                    nc.sync.dma_start(
                        dbg["datt"].rearrange("(t p) q -> p t q", p=P),
                        attn_h[:])
                cur = attn_h
                for wname, bname in (("w1", "b1"), ("w2", "b2"), ("w3", "b3")):
                    nxt = acts.tile([P, DT, NS], F16, tag="y")
                    for ft in range(DT):
                        for h in range(2):
                            ps = pml.tile([P, 512], F32, tag="pml")
                            for dt in range(DT):
                                nc.tensor.matmul(
                                    ps[:],
                                    wT[wname][:, dt, ft * P:(ft + 1) * P],
                                    cur[:, dt, h * 512:(h + 1) * 512],
                                    start=(dt == 0), stop=(dt == DT - 1))
                            nc.scalar.activation(
                                nxt[:, ft, h * 512:(h + 1) * 512], ps[:],
                                AF.Relu, bias=bsb[bname][:, ft:ft + 1])
                    if debug and wname == "w1":
                        nc.sync.dma_start(
                            dbg["dy1"].rearrange("(t p) q -> p t q", p=P),
                            nxt[:])
                    cur = nxt
                for h in range(2):
                    ps = pml.tile([1, 512], F32, tag="pfin")
                    for ft in range(DT):
                        nc.tensor.matmul(
                            ps[:], fwh[:, ft:ft + 1],
                            cur[:, ft, h * 512:(h + 1) * 512],
                            start=(ft == 0), stop=(ft == DT - 1))
                    nc.vector.tensor_copy(out_sb[0:1, h * 512:(h + 1) * 512],
                                          ps[:])
                nc.sync.dma_start(out[:], out_sb[:])
            pacc.release()

    nc.compile()
    return nc


def _get_nc():
    if "nc" not in _CACHE:
        _CACHE["nc"] = _build()
    return _CACHE["nc"]


def _prep_shared(inputs):
    """Host-side prep: transpose + fp16-cast the weights once."""
    names = {"wq": "Wq", "wk": "Wk", "wv": "Wv", "w1": "W1", "w2": "W2",
             "w3": "W3"}
    shared = {}
    for k, v in names.items():
        shared[k + "t"] = np.ascontiguousarray(
            np.asarray(inputs[v], dtype=np.float32).T.astype(np.float16))
    for b in ("bq", "bk", "bv", "b1", "b2", "b3"):
        shared[b] = np.ascontiguousarray(np.asarray(inputs[b],
                                                    dtype=np.float32))
    shared["fw"] = np.ascontiguousarray(
        np.asarray(inputs["final_weight"], dtype=np.float32).reshape(D))
    return shared


def kernel(**inputs):
    nc = _get_nc()
    x = np.asarray(inputs["x"], dtype=np.float32)
    shared = _prep_shared(inputs)
    in_maps = []
    for c in range(NCORES):
        m = dict(shared)
        m["xst"] = np.ascontiguousarray(
            x[c * NS:(c + 1) * NS, :].T.astype(np.float16))
        in_maps.append(m)
    res = bass_utils.run_bass_kernel_spmd(
        nc, in_maps, core_ids=list(range(NCORES)))
    if os.environ.get("K_DEBUG"):
        kernel.debug_results = res.results
    return np.concatenate(
        [res.results[c]["out"].reshape(NS) for c in range(NCORES)])


# revision 30
# speedup vs baseline: 1.4367x; 1.0611x over previous
"""Trainium2 Bass kernel for DeepSelfAttention (N=8192, D=1024) on 8 NeuronCores.

Strategy (row-parallel attention):
  - Shard the N=8192 rows of x across 8 cores (1024 rows each); replicate
    weights. All matmul operands must be contraction-major (features on SBUF
    partitions), so the host pre-transposes and fp16-casts x^T per shard and
    the six d x d weights once in numpy; the device DMAs them straight into
    their final SBUF layouts (no on-device transposes or casts at all).
  - Each core computes Q/K/V projections for its row shard feature-major;
    K^T and V are shipped per key-half: (K^T h0, V h0) -> AllGather0,
    (K^T h1, V h1) -> AllGather1, so the first collective starts as early
    as possible; Q projection fills its latency.
  - Flash-style one-pass attention: scores^T tiles [k=128, q=512] accumulate
    over feature tiles in PSUM, exp on ScalarE (scale=1/32 fused; scores are
    provably in [-3, 3] so no max-subtraction), A@V per (block, dt) with
    free-dim 512 into a rotating set of 4 PSUM banks, flushed to an SBUF
    fp32 accumulator on the DVE; softmax denominator accumulated per
    partition on the DVE and reduced by a single ones-matmul at the end.
  - The V bias is folded into the post-softmax normalize (softmax rows sum
    to 1), where it is a per-partition bias.
  - 3-layer MLP + final projection, feature-major.
DMA queues: bulk loads on Sync, K/V ships on Scalar (they depend on ScalarE
bias-adds anyway), attention block loads on Sync behind the weight loads,
small constants on GpSimd.
All matmul operands are fp16 (full PE rate on TRN2) with fp32 PSUM
accumulation; end-to-end max rel err vs the fp32 reference is ~4e-4.
"""

import os

import numpy as np

import concourse.mybir as mybir
import concourse.tile as tile
from concourse import bacc
from concourse import bass_utils
from concourse.bass import ds

P = 128
D = 1024
N = 8192
NCORES = 8
NS = N // NCORES          # 1024 rows per core
DT = D // P               # 8 feature tiles
KB = 8                    # k blocks (one per source core)
KTB = NS // P             # 8 k tiles per block
KTH = KTB // 2            # 4 k tiles per chunk-block
CH = NS // 2              # 512 keys per chunk
KSZ = D * CH              # K-chunk elements in the flat collective buffer
VSZ = CH * D
F16 = mybir.dt.float16
F32 = mybir.dt.float32
AF = mybir.ActivationFunctionType
ALU = mybir.AluOpType

SCALE = 1.0 / np.sqrt(np.float32(D)).astype(np.float32)  # 0.03125

_CACHE = {}


def _build():
    nc = bacc.Bacc("TRN2", target_bir_lowering=False, debug=False,
                   num_devices=NCORES)
    # host-pretransposed, fp16: x^T shard [D, NS] and W^T [D, D] per weight
    xst = nc.dram_tensor("xst", [D, NS], F16, kind="ExternalInput").ap()
    WT = {}
    for w in ("wq", "wk", "wv", "w1", "w2", "w3"):
        WT[w] = nc.dram_tensor(w + "t", [D, D], F16, kind="ExternalInput").ap()
    B = {}
    for b in ("bq", "bk", "bv", "b1", "b2", "b3"):
        B[b] = nc.dram_tensor(b, [D], F32, kind="ExternalInput").ap()
    fw = nc.dram_tensor("fw", [D], F32, kind="ExternalInput").ap()
    # per-core element offsets of the 7 foreign (K, V) blocks in kvag
    boff = nc.dram_tensor("boff", [1, 14], mybir.dt.uint32,
                          kind="ExternalInput").ap()
    out = nc.dram_tensor("out", [1, NS], F32, kind="ExternalOutput").ap()
    debug = bool(os.environ.get("K_DEBUG"))
    dbg = {}
    if debug:
        for nm, shp, dt_ in (("dq", [D, NS], F16), ("drs", [1, NS], F32),
                             ("datt", [D, NS], F16), ("dy1", [D, NS], F16)):
            dbg[nm] = nc.dram_tensor(nm, shp, dt_, kind="ExternalOutput").ap()

    with tile.TileContext(nc) as tc:
        with (
            tc.tile_pool(name="persist", bufs=1) as pers,
            tc.tile_pool(name="dram", bufs=1, space="DRAM") as dram,
        ):
            # ---- persistent SBUF tiles ----
            qt = pers.tile([P, DT, NS], F16, tag="qt")          # Q^T
            wT = {w: pers.tile([P, DT, D], F16, tag=f"{w}T", name=f"{w}T")
                  for w in ("w1", "w2", "w3")}
            bsb = {b: pers.tile([P, DT], F32, tag=f"{b}sb", name=f"{b}sb")
                   for b in B}
            fwh = pers.tile([P, DT], F16, tag="fwh")
            rs = pers.tile([1, NS], F32, tag="rs")    # softmax denom (debug)

            # ---- DRAM scratch: flat (K-chunk | V-chunk) collective buffers
            kv_d = [dram.tile([KSZ + VSZ], F16, name=f"kv_d{c}")
                    for c in range(2)]
            kvag = [dram.tile([NCORES * (KSZ + VSZ)], F16, name=f"kvag{c}",
                              addr_space="Shared")
                    for c in range(2)]

            # ---- constants (on the GpSimd DMA queue, off the load path) ----
            for b in B:
                nc.gpsimd.dma_start(bsb[b][:],
                                    B[b].rearrange("(t p) -> p t", p=P))
            fwf = pers.tile([P, DT], F32, tag="fwf")
            nc.gpsimd.dma_start(fwf[:], fw.rearrange("(t p) -> p t", p=P))
            nc.vector.tensor_copy(fwh[:], fwf[:])

            # ---- early pool: dies after projections; kvloc (the local
            # K^T/V shard) lives on through the local-block attention ----
            kvloc = tc.alloc_tile_pool(name="kvloc", bufs=1)
            kts = kvloc.tile([P, DT, NS], F16, tag="kts")       # K^T shard
            vs = kvloc.tile([P, KTB, D], F16, tag="vs")         # V shard
            early = tc.alloc_tile_pool(name="early", bufs=1)
            xsT = [early.tile([P, DT, 512], F16, tag=f"xsT{h}",
                              name=f"xsT{h}") for h in range(2)]
            for w in ("wq", "wk", "wv"):
                wT[w] = early.tile([P, DT, D], F16, tag=f"{w}T", name=f"{w}T")

            def load_wt(w):
                nc.sync.dma_start(
                    wT[w][:], WT[w].rearrange("(e p) c -> p e c", p=P))

            with tc.tile_pool(name="ppj", bufs=4, space="PSUM") as ppj:
                # loads ordered to unblock K h0, V h0 fastest
                nc.sync.dma_start(
                    xsT[0][:],
                    xst[:, 0:512].rearrange("(e p) n -> p e n", p=P))
                load_wt("wk")
                load_wt("wv")
                nc.sync.dma_start(
                    xsT[1][:],
                    xst[:, 512:1024].rearrange("(e p) n -> p e n", p=P))
                load_wt("wq")
                for w in ("w1", "w2", "w3"):
                    load_wt(w)

                def kv_half(h):
                    # K^T[:, half] = Wk @ xs^T + bk
                    for dt in range(DT):
                        ps = ppj.tile([P, 512], F32, tag="ppj")
                        for et in range(DT):
                            nc.tensor.matmul(
                                ps[:],
                                wT["wk"][:, et, dt * P:(dt + 1) * P],
                                xsT[h][:, et, :],
                                start=(et == 0), stop=(et == DT - 1))
                        nc.scalar.activation(
                            kts[:, dt, h * 512:(h + 1) * 512], ps[:],
                            AF.Identity, bias=bsb["bk"][:, dt:dt + 1])
                    nc.scalar.dma_start(
                        kv_d[h][0:KSZ].rearrange("(p t k) -> p t k", p=P, k=CH),
                        kts[:, :, h * CH:(h + 1) * CH])
                    # V[half] = xs @ Wv.T (bias folded into the normalize)
                    for kt in range(h * KTH, (h + 1) * KTH):
                        for dh in range(2):
                            ps = ppj.tile([P, 512], F32, tag="ppj")
                            for et in range(DT):
                                nc.tensor.matmul(
                                    ps[:],
                                    xsT[h][:, et,
                                           (kt - h * KTH) * P:
                                           (kt - h * KTH + 1) * P],
                                    wT["wv"][:, et, dh * 512:(dh + 1) * 512],
                                    start=(et == 0), stop=(et == DT - 1))
                            nc.scalar.copy(
                                vs[:, kt, dh * 512:(dh + 1) * 512], ps[:])
                    nc.scalar.dma_start(
                        kv_d[h][KSZ:].rearrange("(p t d) -> p t d", p=P, d=D),
                        vs[:, h * KTH:(h + 1) * KTH, :])
                    nc.gpsimd.collective_compute(
                        "AllGather", ALU.bypass,
                        replica_groups=[list(range(NCORES))],
                        ins=[kv_d[h].opt()], outs=[kvag[h].opt()])

                def q_half(h):
                    for dt in range(DT):
                        ps = ppj.tile([P, 512], F32, tag="ppj")
                        for et in range(DT):
                            nc.tensor.matmul(
                                ps[:],
                                wT["wq"][:, et, dt * P:(dt + 1) * P],
                                xsT[h][:, et, :],
                                start=(et == 0), stop=(et == DT - 1))
                        nc.scalar.activation(
                            qt[:, dt, h * 512:(h + 1) * 512], ps[:],
                            AF.Identity, bias=bsb["bq"][:, dt:dt + 1])

                # K/V half0 -> AllGather0; Q-projection fills AllGather0's
                # latency; then K/V half1 -> AllGather1 (still well before
                # chunk-1 attention needs it)
                kv_half(0)
                q_half(0)
                q_half(1)
                kv_half(1)

            early.release()

            if debug:
                nc.sync.dma_start(dbg["dq"].rearrange("(t p) k -> p t k", p=P),
                                  qt[:])

            # ---- attention: 2 local blocks (from SBUF, during AllGather0)
            # then 2 chunks x 7 foreign blocks via per-core dynamic offsets
            pacc = tc.alloc_tile_pool(name="pacc", bufs=1)
            attacc = pacc.tile([P, DT, NS], F32, tag="attacc")
            rs_acc = pacc.tile([P, 2, 512], F32, tag="rs_acc")
            osb = pacc.tile([1, 14], mybir.dt.uint32, tag="osb")
            nc.gpsimd.dma_start(osb[:], boff)
            with (
                tc.tile_pool(name="kv", bufs=3) as kv,
                tc.tile_pool(name="ex", bufs=8) as exp_pool,
                tc.tile_pool(name="psc", bufs=2, space="PSUM") as psc,
                tc.tile_pool(name="pat", bufs=4, space="PSUM") as pat,
            ):
                def attn_block(kt_tile, kt_base, v_tile, v_ktbase, first):
                    for qp in range(2):
                        qpsl = slice(qp * 512, (qp + 1) * 512)
                        exs = []
                        for kt in range(KTH):
                            sc = psc.tile([P, 512], F32, tag="psc")
                            for dt in range(DT):
                                nc.tensor.matmul(
                                    sc[:],
                                    kt_tile[:, dt,
                                            kt_base + kt * P:
                                            kt_base + (kt + 1) * P],
                                    qt[:, dt, qpsl],
                                    start=(dt == 0), stop=(dt == DT - 1))
                            ex = exp_pool.tile([P, 512], F16, tag="ex",
                                               name=f"ex{kt}")
                            nc.scalar.activation(ex[:], sc[:], AF.Exp,
                                                 scale=float(SCALE))
                            # softmax denominator: per-partition partial
                            # sums on the DVE (reduced at the end)
                            if first and kt == 0:
                                nc.vector.tensor_copy(rs_acc[:, qp, :], ex[:])
                            else:
                                nc.vector.tensor_tensor(
                                    rs_acc[:, qp, :], ex[:],
                                    rs_acc[:, qp, :], ALU.add)
                            exs.append(ex)
                        # A@V: per dt, accumulate the 4 kt matmuls in one
                        # PSUM bank (free dim 512), 4 banks rotating
                        for dt in range(DT):
                            att_ps = pat.tile([P, 512], F32, tag="pat")
                            for kt in range(KTH):
                                nc.tensor.matmul(
                                    att_ps[:],
                                    v_tile[:, v_ktbase + kt,
                                           dt * P:(dt + 1) * P],
                                    exs[kt][:],
                                    start=(kt == 0),
                                    stop=(kt == KTH - 1))
                            dsl = (slice(None), dt, qpsl)
                            if first:
                                nc.vector.tensor_copy(attacc[dsl], att_ps[:])
                            else:
                                nc.vector.tensor_tensor(
                                    attacc[dsl], att_ps[:],
                                    attacc[dsl], ALU.add)

                # foreign-block offsets into registers (Sync engine issues
                # the gathered-block DMAs)
                BLK = KSZ + VSZ
                kofs, vofs = [], []
                for j in range(7):
                    rk = nc.sync.alloc_register(f"koff{j}")
                    nc.sync.reg_load(rk, osb[0:1, j:j + 1])
                    kofs.append(nc.sync.snap(rk, donate=True, min_val=0,
                                             max_val=7 * BLK))
                    rv = nc.sync.alloc_register(f"voff{j}")
                    nc.sync.reg_load(rv, osb[0:1, 7 + j:8 + j])
                    vofs.append(nc.sync.snap(rv, donate=True, min_val=0,
                                             max_val=7 * BLK + KSZ))

                # local blocks: K^T/V already in SBUF; runs under AllGather0
                attn_block(kts, 0, vs, 0, True)
                attn_block(kts, CH, vs, KTH, False)

                for ch in range(2):
                    flat = kvag[ch]
                    for j in range(7):
                        ktb = kv.tile([P, DT, CH], F16, tag="ktb")
                        vb = kv.tile([P, KTH, D], F16, tag="vb")
                        nc.sync.dma_start(
                            ktb[:],
                            flat[ds(kofs[j], KSZ)].rearrange(
                                "(p t k) -> p t k", p=P, k=CH),
                            bounds_check="err")
                        nc.sync.dma_start(
                            vb[:],
                            flat[ds(vofs[j], VSZ)].rearrange(
                                "(p t d) -> p t d", p=P, d=D),
                            bounds_check="err")
                        attn_block(ktb, 0, vb, 0, False)
                # reduce rs_acc across partitions, replicated to all 128
                # partitions (ones [P,P] stationary), then a parallel
                # reciprocal straight into the normalize operand
                recip_b = pacc.tile([P, 2, 512], F32, tag="recip_b")
                with tc.tile_pool(name="prs", bufs=2, space="PSUM") as prs:
                    ones_f = pacc.tile([P, P], F32, tag="ones_f")
                    nc.gpsimd.memset(ones_f[:], 1.0)
                    for qp in range(2):
                        rs_ps = prs.tile([P, 512], F32, tag="prs")
                        nc.tensor.matmul(rs_ps[:], ones_f[:],
                                         rs_acc[:, qp, :])
                        nc.vector.reciprocal(recip_b[:, qp, :], rs_ps[:])
                        if debug:
                            nc.vector.tensor_copy(
                                rs[0:1, qp * 512:(qp + 1) * 512],
                                rs_ps[0:1, :])

            # ---- normalize + MLP + final ----
            with (
                tc.tile_pool(name="acts", bufs=2) as acts,
                tc.tile_pool(name="pml", bufs=4, space="PSUM") as pml,
            ):
                out_sb = acts.tile([1, NS], F32, tag="out_sb")
                attn_h = acts.tile([P, DT, NS], F16, tag="y")
                for h in range(2):
                    qsl = slice(h * 512, (h + 1) * 512)
                    for dt in range(DT):
                        nc.vector.tensor_tensor(
                            attn_h[:, dt, qsl], attacc[:, dt, qsl],
                            recip_b[:, h, :], ALU.mult)
                        nc.vector.tensor_tensor(
                            attn_h[:, dt, qsl], attn_h[:, dt, qsl],
                            bsb["bv"][:, dt:dt + 1].to_broadcast([P, 512]),
                            ALU.add)
                if debug:
                    nc.sync.dma_start(dbg["drs"][:], rs[:])
                    nc.sync.dma_start(
                        dbg["datt"].rearrange("(t p) q -> p t q", p=P),
                        attn_h[:])
                cur = attn_h
                for wname, bname in (("w1", "b1"), ("w2", "b2"), ("w3", "b3")):
                    nxt = acts.tile([P, DT, NS], F16, tag="y")
                    for ft in range(DT):
                        for h in range(2):
                            ps = pml.tile([P, 512], F32, tag="pml")
                            for dt in range(DT):
                                nc.tensor.matmul(
                                    ps[:],
                                    wT[wname][:, dt, ft * P:(ft + 1) * P],
                                    cur[:, dt, h * 512:(h + 1) * 512],
                                    start=(dt == 0), stop=(dt == DT - 1))
                            nc.scalar.activation(
                                nxt[:, ft, h * 512:(h + 1) * 512], ps[:],
                                AF.Relu, bias=bsb[bname][:, ft:ft + 1])
                    if debug and wname == "w1":
                        nc.sync.dma_start(
                            dbg["dy1"].rearrange("(t p) q -> p t q", p=P),
                            nxt[:])
                    cur = nxt
                for h in range(2):
                    ps = pml.tile([1, 512], F32, tag="pfin")
                    for ft in range(DT):
                        nc.tensor.matmul(
                            ps[:], fwh[:, ft:ft + 1],
                            cur[:, ft, h * 512:(h + 1) * 512],
                            start=(ft == 0), stop=(ft == DT - 1))
                    nc.vector.tensor_copy(out_sb[0:1, h * 512:(h + 1) * 512],
                                          ps[:])
                nc.sync.dma_start(out[:], out_sb[:])
            pacc.release()
            kvloc.release()

    nc.compile()
    return nc


def _get_nc():
    if "nc" not in _CACHE:
        _CACHE["nc"] = _build()
    return _CACHE["nc"]


def _prep_shared(inputs):
    """Host-side prep: transpose + fp16-cast the weights once."""
    names = {"wq": "Wq", "wk": "Wk", "wv": "Wv", "w1": "W1", "w2": "W2",
             "w3": "W3"}
    shared = {}
    for k, v in names.items():
        shared[k + "t"] = np.ascontiguousarray(
            np.asarray(inputs[v], dtype=np.float32).T.astype(np.float16))
    for b in ("bq", "bk", "bv", "b1", "b2", "b3"):
        shared[b] = np.ascontiguousarray(np.asarray(inputs[b],
                                                    dtype=np.float32))
    shared["fw"] = np.ascontiguousarray(
        np.asarray(inputs["final_weight"], dtype=np.float32).reshape(D))
    return shared


def _boff(c):
    """Element offsets of the 7 foreign (K, V) blocks for core c."""
    blk = KSZ + VSZ
    ks = [((c + 1 + j) % NCORES) * blk for j in range(7)]
    return np.array([ks + [k + KSZ for k in ks]], dtype=np.uint32)


def kernel(**inputs):
    nc = _get_nc()
    x = np.asarray(inputs["x"], dtype=np.float32)
    shared = _prep_shared(inputs)
    in_maps = []
    for c in range(NCORES):
        m = dict(shared)
        m["xst"] = np.ascontiguousarray(
            x[c * NS:(c + 1) * NS, :].T.astype(np.float16))
        m["boff"] = _boff(c)
        in_maps.append(m)
    res = bass_utils.run_bass_kernel_spmd(
        nc, in_maps, core_ids=list(range(NCORES)))
    if os.environ.get("K_DEBUG"):
        kernel.debug_results = res.results
    return np.concatenate(
        [res.results[c]["out"].reshape(NS) for c in range(NCORES)])


# revision 32
# speedup vs baseline: 1.4423x; 1.0039x over previous
"""Trainium2 Bass kernel for DeepSelfAttention (N=8192, D=1024) on 8 NeuronCores.

Strategy (row-parallel attention):
  - Shard the N=8192 rows of x across 8 cores (1024 rows each); replicate
    weights. The host pre-transposes and fp16-casts x^T per shard and the
    weights once in numpy; the device DMAs them straight into their final
    SBUF layouts (no on-device transposes or casts).
  - Softmax is invariant to per-row constants, so the QK^T scores reduce to
    U @ xs_all^T with U = xs @ G + bq@Wk, G = Wq^T Wk (host-precomputed).
    This removes the K projection entirely, and the "keys" operand of the
    score matmuls is raw x^T — so the first AllGather (of x^T) is triggered
    at t=0, before any compute. V halves follow on two more AllGathers.
  - Flash-style one-pass attention: scores^T tiles [k=128, q=512] accumulate
    over feature tiles in PSUM, exp on ScalarE (scale=1/32 fused; scores are
    provably in [-3, 3] so no max-subtraction), A@V per (block, dt) with
    free-dim 512 into a rotating set of 4 PSUM banks, flushed to an SBUF
    fp32 accumulator on the DVE; softmax denominator accumulated per
    partition on the DVE and reduced by a single ones-matmul at the end.
  - The two local blocks run from SBUF during the x^T AllGather; the 14
    foreign blocks are fetched with per-core dynamic DMA offsets (skipping
    the core's own slot in the gathered buffers).
  - The V bias is folded into the MLP's first-layer bias on the host
    (b1' = b1 + W1 @ bv); 3-layer MLP + final projection, feature-major.
All matmul operands are fp16 (full PE rate on TRN2) with fp32 PSUM
accumulation; end-to-end max rel err vs the fp32 reference is ~4e-4.
"""

import os

import numpy as np

import concourse.mybir as mybir
import concourse.tile as tile
from concourse import bacc
from concourse import bass_utils
from concourse.bass import ds

P = 128
D = 1024
N = 8192
NCORES = 8
NS = N // NCORES          # 1024 rows per core
DT = D // P               # 8 feature tiles
KTB = NS // P             # 8 k tiles per block
KTH = KTB // 2            # 4 k tiles per chunk-block
CH = NS // 2              # 512 keys per chunk
XN = P * 2 * DT * 512     # x^T shard elements (= D * NS)
VSZ = CH * D              # V-chunk elements
F16 = mybir.dt.float16
F32 = mybir.dt.float32
AF = mybir.ActivationFunctionType
ALU = mybir.AluOpType

SCALE = 1.0 / np.sqrt(np.float32(D)).astype(np.float32)  # 0.03125

_CACHE = {}


def _build():
    nc = bacc.Bacc("TRN2", target_bir_lowering=False, debug=False,
                   num_devices=NCORES)
    # x^T shard, host-packed [p, chunk, e, n'] so every partition line is
    # contiguous per chunk (8 KB DMA descriptors)
    xst = nc.dram_tensor("xst", [P, 2, DT, 512], F16,
                         kind="ExternalInput").ap()
    # host-precomputed G = Wq^T Wk and Wv^T, fp16
    g = nc.dram_tensor("g", [D, D], F16, kind="ExternalInput").ap()
    wvt = nc.dram_tensor("wvt", [D, D], F16, kind="ExternalInput").ap()
    WT = {"w1": None, "w2": None, "w3": None}
    for w in WT:
        WT[w] = nc.dram_tensor(w + "t", [D, D], F16, kind="ExternalInput").ap()
    B = {}
    for b in ("ub", "b1", "b2", "b3"):
        B[b] = nc.dram_tensor(b, [D], F32, kind="ExternalInput").ap()
    fw = nc.dram_tensor("fw", [D], F32, kind="ExternalInput").ap()
    out = nc.dram_tensor("out", [1, NS], F32, kind="ExternalOutput").ap()
    # per-core element offsets of the 7 foreign (x^T, V) blocks
    boff = nc.dram_tensor("boff", [1, 14], mybir.dt.uint32,
                          kind="ExternalInput").ap()
    debug = bool(os.environ.get("K_DEBUG"))
    dbg = {}
    if debug:
        for nm, shp, dt_ in (("dq", [D, NS], F16), ("drs", [1, NS], F32),
                             ("datt", [D, NS], F16), ("dy1", [D, NS], F16)):
            dbg[nm] = nc.dram_tensor(nm, shp, dt_, kind="ExternalOutput").ap()

    with tile.TileContext(nc) as tc:
        with (
            tc.tile_pool(name="persist", bufs=1) as pers,
            tc.tile_pool(name="dram", bufs=1, space="DRAM") as dram,
        ):
            # ---- persistent SBUF tiles ----
            qt = pers.tile([P, DT, NS], F16, tag="qt")          # U^T
            wT = {w: pers.tile([P, DT, D], F16, tag=f"{w}T", name=f"{w}T")
                  for w in ("w1", "w2", "w3")}
            bsb = {b: pers.tile([P, DT], F32, tag=f"{b}sb", name=f"{b}sb")
                   for b in B}
            fwh = pers.tile([P, DT], F16, tag="fwh")
            rs = pers.tile([1, NS], F32, tag="rs")    # softmax denom (debug)

            # ---- DRAM scratch: collective buffers ----
            kv_d = [dram.tile([VSZ], F16, name=f"kv_d{c}") for c in range(2)]
            x_d = dram.tile([XN], F16, name="x_d")
            kvag_x = dram.tile([NCORES * XN], F16, name="kvag_x",
                               addr_space="Shared")
            kvag_v = [dram.tile([NCORES * VSZ], F16, name=f"kvag_v{c}",
                                addr_space="Shared")
                      for c in range(2)]

            # x^T AllGather at t=0 (collectives can't read IO tensors, so
            # bounce the input through a DRAM scratch copy first)
            nc.sync.dma_start(x_d[:], xst.rearrange("p c e n -> (p c e n)"))
            nc.gpsimd.collective_compute(
                "AllGather", ALU.bypass,
                replica_groups=[list(range(NCORES))],
                ins=[x_d.opt()], outs=[kvag_x.opt()])

            # ---- constants (on the GpSimd DMA queue, off the load path) ----
            for b in B:
                nc.gpsimd.dma_start(bsb[b][:],
                                    B[b].rearrange("(t p) -> p t", p=P))
            fwf = pers.tile([P, DT], F32, tag="fwf")
            nc.gpsimd.dma_start(fwf[:], fw.rearrange("(t p) -> p t", p=P))
            nc.vector.tensor_copy(fwh[:], fwf[:])

            # ---- kvloc: local x^T halves + V shard, live through the
            # local-block attention; early: wv/g weights ----
            kvloc = tc.alloc_tile_pool(name="kvloc", bufs=1)
            xsT = [kvloc.tile([P, DT, 512], F16, tag=f"xsT{h}",
                              name=f"xsT{h}") for h in range(2)]
            vs = kvloc.tile([P, KTB, D], F16, tag="vs")         # V shard
            early = tc.alloc_tile_pool(name="early", bufs=1)
            wvT = early.tile([P, DT, D], F16, tag="wvT")
            gT = early.tile([P, DT, D], F16, tag="gT")

            with tc.tile_pool(name="ppj", bufs=4, space="PSUM") as ppj:
                nc.sync.dma_start(xsT[0][:], xst[:, 0])
                nc.sync.dma_start(
                    wvT[:], wvt.rearrange("(e p) c -> p e c", p=P))
                nc.sync.dma_start(xsT[1][:], xst[:, 1])
                nc.sync.dma_start(gT[:], g.rearrange("(e p) c -> p e c", p=P))
                for w in ("w1", "w2", "w3"):
                    nc.sync.dma_start(
                        wT[w][:], WT[w].rearrange("(e p) c -> p e c", p=P))

                # V halves -> ship -> AllGather (queued behind the x gather)
                for h in range(2):
                    for kt in range(h * KTH, (h + 1) * KTH):
                        for dh in range(2):
                            ps = ppj.tile([P, 512], F32, tag="ppj")
                            for et in range(DT):
                                nc.tensor.matmul(
                                    ps[:],
                                    xsT[h][:, et,
                                           (kt - h * KTH) * P:
                                           (kt - h * KTH + 1) * P],
                                    wvT[:, et, dh * 512:(dh + 1) * 512],
                                    start=(et == 0), stop=(et == DT - 1))
                            nc.scalar.copy(
                                vs[:, kt, dh * 512:(dh + 1) * 512], ps[:])
                    nc.scalar.dma_start(
                        kv_d[h].rearrange("(p t d) -> p t d", p=P, d=D),
                        vs[:, h * KTH:(h + 1) * KTH, :])
                    nc.gpsimd.collective_compute(
                        "AllGather", ALU.bypass,
                        replica_groups=[list(range(NCORES))],
                        ins=[kv_d[h].opt()], outs=[kvag_v[h].opt()])

                # U^T = G^T @ xs^T + (bq Wk): fills the collective latency
                for dt in range(DT):
                    for h in range(2):
                        ps = ppj.tile([P, 512], F32, tag="ppj")
                        for et in range(DT):
                            nc.tensor.matmul(
                                ps[:],
                                gT[:, et, dt * P:(dt + 1) * P],
                                xsT[h][:, et, :],
                                start=(et == 0), stop=(et == DT - 1))
                        nc.scalar.activation(
                            qt[:, dt, h * 512:(h + 1) * 512], ps[:],
                            AF.Identity, bias=bsb["ub"][:, dt:dt + 1])

            early.release()

            if debug:
                nc.sync.dma_start(dbg["dq"].rearrange("(t p) k -> p t k", p=P),
                                  qt[:])

            # ---- attention: 2 local blocks (from SBUF, during AllGathers)
            # then 2 chunks x 7 foreign blocks via per-core dynamic offsets
            pacc = tc.alloc_tile_pool(name="pacc", bufs=1)
            attacc = pacc.tile([P, DT, NS], F32, tag="attacc")
            rs_acc = pacc.tile([P, 2, 512], F32, tag="rs_acc")
            osb = pacc.tile([1, 14], mybir.dt.uint32, tag="osb")
            nc.gpsimd.dma_start(osb[:], boff)
            with (
                tc.tile_pool(name="kv", bufs=3) as kv,
                tc.tile_pool(name="ex", bufs=8) as exp_pool,
                tc.tile_pool(name="psc", bufs=2, space="PSUM") as psc,
                tc.tile_pool(name="pat", bufs=4, space="PSUM") as pat,
            ):
                def attn_block(kt_tile, v_tile, v_ktbase, first):
                    for qp in range(2):
                        qpsl = slice(qp * 512, (qp + 1) * 512)
                        exs = []
                        for kt in range(KTH):
                            sc = psc.tile([P, 512], F32, tag="psc")
                            for dt in range(DT):
                                nc.tensor.matmul(
                                    sc[:],
                                    kt_tile[:, dt, kt * P:(kt + 1) * P],
                                    qt[:, dt, qpsl],
                                    start=(dt == 0), stop=(dt == DT - 1))
                            ex = exp_pool.tile([P, 512], F16, tag="ex",
                                               name=f"ex{kt}")
                            nc.scalar.activation(ex[:], sc[:], AF.Exp,
                                                 scale=float(SCALE))
                            # softmax denominator: per-partition partial
                            # sums on the DVE (reduced at the end)
                            if first and kt == 0:
                                nc.vector.tensor_copy(rs_acc[:, qp, :], ex[:])
                            else:
                                nc.vector.tensor_tensor(
                                    rs_acc[:, qp, :], ex[:],
                                    rs_acc[:, qp, :], ALU.add)
                            exs.append(ex)
                        # A@V: per dt, accumulate the 4 kt matmuls in one
                        # PSUM bank (free dim 512), 4 banks rotating
                        for dt in range(DT):
                            att_ps = pat.tile([P, 512], F32, tag="pat")
                            for kt in range(KTH):
                                nc.tensor.matmul(
                                    att_ps[:],
                                    v_tile[:, v_ktbase + kt,
                                           dt * P:(dt + 1) * P],
                                    exs[kt][:],
                                    start=(kt == 0),
                                    stop=(kt == KTH - 1))
                            dsl = (slice(None), dt, qpsl)
                            if first:
                                nc.vector.tensor_copy(attacc[dsl], att_ps[:])
                            else:
                                nc.vector.tensor_tensor(
                                    attacc[dsl], att_ps[:],
                                    attacc[dsl], ALU.add)

                # foreign-block offsets into registers (Sync engine issues
                # the gathered-block DMAs)
                kofs, vofs = [], []
                for j in range(7):
                    rk = nc.sync.alloc_register(f"koff{j}")
                    nc.sync.reg_load(rk, osb[0:1, j:j + 1])
                    kofs.append(nc.sync.snap(rk, donate=True, min_val=0,
                                             max_val=7 * XN))
                    rv = nc.sync.alloc_register(f"voff{j}")
                    nc.sync.reg_load(rv, osb[0:1, 7 + j:8 + j])
                    vofs.append(nc.sync.snap(rv, donate=True, min_val=0,
                                             max_val=7 * VSZ))

                # local blocks: x^T/V already in SBUF; run under AllGather0
                attn_block(xsT[0], vs, 0, True)
                attn_block(xsT[1], vs, KTH, False)

                for ch in range(2):
                    for j in range(7):
                        ktb = kv.tile([P, DT, CH], F16, tag="ktb")
                        vb = kv.tile([P, KTH, D], F16, tag="vb")
                        nc.sync.dma_start(
                            ktb[:],
                            kvag_x[ds(kofs[j], XN)].rearrange(
                                "(p c e n) -> p c e n",
                                p=P, c=2, e=DT)[:, ch],
                            bounds_check="err")
                        nc.sync.dma_start(
                            vb[:],
                            kvag_v[ch][ds(vofs[j], VSZ)].rearrange(
                                "(p t d) -> p t d", p=P, d=D),
                            bounds_check="err")
                        attn_block(ktb, vb, 0, False)
                # reduce rs_acc across partitions, replicated to all 128
                # partitions (ones [P,P] stationary), then a parallel
                # reciprocal straight into the normalize operand
                recip_b = pacc.tile([P, 2, 512], F32, tag="recip_b")
                with tc.tile_pool(name="prs", bufs=2, space="PSUM") as prs:
                    ones_f = pacc.tile([P, P], F32, tag="ones_f")
                    nc.gpsimd.memset(ones_f[:], 1.0)
                    for qp in range(2):
                        rs_ps = prs.tile([P, 512], F32, tag="prs")
                        nc.tensor.matmul(rs_ps[:], ones_f[:],
                                         rs_acc[:, qp, :])
                        nc.vector.reciprocal(recip_b[:, qp, :], rs_ps[:])
                        if debug:
                            nc.vector.tensor_copy(
                                rs[0:1, qp * 512:(qp + 1) * 512],
                                rs_ps[0:1, :])

            # ---- normalize + MLP + final (V bias folded into b1) ----
            with (
                tc.tile_pool(name="acts", bufs=2) as acts,
                tc.tile_pool(name="pml", bufs=4, space="PSUM") as pml,
            ):
                out_sb = acts.tile([1, NS], F32, tag="out_sb")
                attn_h = acts.tile([P, DT, NS], F16, tag="y")
                for h in range(2):
                    qsl = slice(h * 512, (h + 1) * 512)
                    for dt in range(DT):
                        nc.vector.tensor_tensor(
                            attn_h[:, dt, qsl], attacc[:, dt, qsl],
                            recip_b[:, h, :], ALU.mult)
                if debug:
                    nc.sync.dma_start(dbg["drs"][:], rs[:])
                    nc.sync.dma_start(
                        dbg["datt"].rearrange("(t p) q -> p t q", p=P),
                        attn_h[:])
                cur = attn_h
                for wname, bname in (("w1", "b1"), ("w2", "b2"), ("w3", "b3")):
                    nxt = acts.tile([P, DT, NS], F16, tag="y")
                    for ft in range(DT):
                        for h in range(2):
                            ps = pml.tile([P, 512], F32, tag="pml")
                            for dt in range(DT):
                                nc.tensor.matmul(
                                    ps[:],
                                    wT[wname][:, dt, ft * P:(ft + 1) * P],
                                    cur[:, dt, h * 512:(h + 1) * 512],
                                    start=(dt == 0), stop=(dt == DT - 1))
                            nc.scalar.activation(
                                nxt[:, ft, h * 512:(h + 1) * 512], ps[:],
                                AF.Relu, bias=bsb[bname][:, ft:ft + 1])
                    if debug and wname == "w1":
                        nc.sync.dma_start(
                            dbg["dy1"].rearrange("(t p) q -> p t q", p=P),
                            nxt[:])
                    cur = nxt
                for h in range(2):
                    ps = pml.tile([1, 512], F32, tag="pfin")
                    for ft in range(DT):
                        nc.tensor.matmul(
                            ps[:], fwh[:, ft:ft + 1],
                            cur[:, ft, h * 512:(h + 1) * 512],
                            start=(ft == 0), stop=(ft == DT - 1))
                    nc.vector.tensor_copy(out_sb[0:1, h * 512:(h + 1) * 512],
                                          ps[:])
                nc.sync.dma_start(out[:], out_sb[:])
            pacc.release()
            kvloc.release()

    nc.compile()
    return nc


def _get_nc():
    if "nc" not in _CACHE:
        _CACHE["nc"] = _build()
    return _CACHE["nc"]


def _prep_shared(inputs):
    """Host-side prep: fold/transform the weights once in numpy."""
    f32 = np.float32
    Wq = np.asarray(inputs["Wq"], f32)
    Wk = np.asarray(inputs["Wk"], f32)
    Wv = np.asarray(inputs["Wv"], f32)
    W1 = np.asarray(inputs["W1"], f32)
    shared = {
        "g": np.ascontiguousarray((Wq.T @ Wk).astype(np.float16)),
        "wvt": np.ascontiguousarray(Wv.T.astype(np.float16)),
        "w1t": np.ascontiguousarray(W1.T.astype(np.float16)),
        "w2t": np.ascontiguousarray(
            np.asarray(inputs["W2"], f32).T.astype(np.float16)),
        "w3t": np.ascontiguousarray(
            np.asarray(inputs["W3"], f32).T.astype(np.float16)),
        "ub": np.ascontiguousarray(np.asarray(inputs["bq"], f32) @ Wk),
        "b1": np.ascontiguousarray(
            np.asarray(inputs["b1"], f32)
            + W1 @ np.asarray(inputs["bv"], f32)),
        "b2": np.ascontiguousarray(np.asarray(inputs["b2"], f32)),
        "b3": np.ascontiguousarray(np.asarray(inputs["b3"], f32)),
        "fw": np.ascontiguousarray(
            np.asarray(inputs["final_weight"], f32).reshape(D)),
    }
    return shared


def _boff(c):
    """Element offsets of the 7 foreign (x^T, V) blocks for core c."""
    blks = [(c + 1 + j) % NCORES for j in range(7)]
    ks = [b * XN for b in blks]
    vso = [b * VSZ for b in blks]
    return np.array([ks + vso], dtype=np.uint32)


def _pack_xst(x_shard):
    """[NS, D] fp32 -> [P, 2, DT, 512] fp16, x^T packed chunk-major."""
    a = x_shard.T.astype(np.float16)           # [D, NS]
    a = a.reshape(DT, P, 2, 512)               # [e, p, ch, n']
    return np.ascontiguousarray(a.transpose(1, 2, 0, 3))


def kernel(**inputs):
    nc = _get_nc()
    x = np.asarray(inputs["x"], dtype=np.float32)
    shared = _prep_shared(inputs)
    in_maps = []
    for c in range(NCORES):
        m = dict(shared)
        m["xst"] = _pack_xst(x[c * NS:(c + 1) * NS, :])
        m["boff"] = _boff(c)
        in_maps.append(m)
    res = bass_utils.run_bass_kernel_spmd(
        nc, in_maps, core_ids=list(range(NCORES)))
    if os.environ.get("K_DEBUG"):
        kernel.debug_results = res.results
    return np.concatenate(
        [res.results[c]["out"].reshape(NS) for c in range(NCORES)])


# revision 34
# speedup vs baseline: 1.4458x; 1.0025x over previous
"""Trainium2 Bass kernel for DeepSelfAttention (N=8192, D=1024) on 8 NeuronCores.

Strategy (row-parallel attention):
  - Shard the N=8192 rows of x across 8 cores (1024 rows each); replicate
    weights. The host pre-transposes and fp16-casts x^T per shard and the
    weights once in numpy; the device DMAs them straight into their final
    SBUF layouts (no on-device transposes or casts).
  - Softmax is invariant to per-row constants, so the QK^T scores reduce to
    U @ xs_all^T with U = xs @ G + bq@Wk, G = Wq^T Wk (host-precomputed).
    This removes the K projection entirely, and the "keys" operand of the
    score matmuls is raw x^T — so the first AllGather (of x^T) is triggered
    at t=0, before any compute. V halves follow on two more AllGathers.
  - Flash-style one-pass attention: scores^T tiles [k=128, q=512] accumulate
    over feature tiles in PSUM, exp on ScalarE (scale=1/32 fused; scores are
    provably in [-3, 3] so no max-subtraction), A@V per (block, dt) with
    free-dim 512 into a rotating set of 4 PSUM banks, flushed to an SBUF
    fp32 accumulator on the DVE; softmax denominator accumulated per
    partition on the DVE and reduced by a single ones-matmul at the end.
  - The two local blocks run from SBUF during the x^T AllGather; the 14
    foreign blocks are fetched with per-core dynamic DMA offsets (skipping
    the core's own slot in the gathered buffers).
  - The V bias is folded into the MLP's first-layer bias on the host
    (b1' = b1 + W1 @ bv); 3-layer MLP + final projection, feature-major.
All matmul operands are fp16 (full PE rate on TRN2) with fp32 PSUM
accumulation; end-to-end max rel err vs the fp32 reference is ~4e-4.
"""

import os

import numpy as np

import concourse.mybir as mybir
import concourse.tile as tile
from concourse import bacc
from concourse import bass_utils
from concourse.bass import ds

P = 128
D = 1024
N = 8192
NCORES = 8
NS = N // NCORES          # 1024 rows per core
DT = D // P               # 8 feature tiles
KTB = NS // P             # 8 k tiles per block
KTH = KTB // 2            # 4 k tiles per chunk-block
CH = NS // 2              # 512 keys per chunk
XN = P * 2 * DT * 512     # x^T shard elements (= D * NS)
VSZ = CH * D              # V-chunk elements
F16 = mybir.dt.float16
F32 = mybir.dt.float32
AF = mybir.ActivationFunctionType
ALU = mybir.AluOpType

SCALE = 1.0 / np.sqrt(np.float32(D)).astype(np.float32)  # 0.03125

_CACHE = {}


def _build():
    nc = bacc.Bacc("TRN2", target_bir_lowering=False, debug=False,
                   num_devices=NCORES)
    # x^T shard, host-packed [p, chunk, e, n'] so every partition line is
    # contiguous per chunk (8 KB DMA descriptors)
    xst = nc.dram_tensor("xst", [P, 2, DT, 512], F16,
                         kind="ExternalInput").ap()
    # host-precomputed G = Wq^T Wk and Wv^T, fp16
    g = nc.dram_tensor("g", [D, D], F16, kind="ExternalInput").ap()
    wvt = nc.dram_tensor("wvt", [D, D], F16, kind="ExternalInput").ap()
    WT = {"w1": None, "w2": None, "w3": None}
    for w in WT:
        WT[w] = nc.dram_tensor(w + "t", [D, D], F16, kind="ExternalInput").ap()
    B = {}
    for b in ("ub", "b1", "b2", "b3"):
        B[b] = nc.dram_tensor(b, [D], F32, kind="ExternalInput").ap()
    fw = nc.dram_tensor("fw", [D], F32, kind="ExternalInput").ap()
    out = nc.dram_tensor("out", [1, NS], F32, kind="ExternalOutput").ap()
    # per-core element offsets of the 7 foreign (x^T, V) blocks
    boff = nc.dram_tensor("boff", [1, 14], mybir.dt.uint32,
                          kind="ExternalInput").ap()
    debug = bool(os.environ.get("K_DEBUG"))
    dbg = {}
    if debug:
        for nm, shp, dt_ in (("dq", [D, NS], F16), ("drs", [1, NS], F32),
                             ("datt", [D, NS], F16), ("dy1", [D, NS], F16)):
            dbg[nm] = nc.dram_tensor(nm, shp, dt_, kind="ExternalOutput").ap()

    with tile.TileContext(nc) as tc:
        with (
            tc.tile_pool(name="persist", bufs=1) as pers,
            tc.tile_pool(name="dram", bufs=1, space="DRAM") as dram,
        ):
            # ---- persistent SBUF tiles ----
            qt = pers.tile([P, DT, NS], F16, tag="qt")          # U^T
            wT = {w: pers.tile([P, DT, D], F16, tag=f"{w}T", name=f"{w}T")
                  for w in ("w1", "w2", "w3")}
            bsb = {b: pers.tile([P, DT], F32, tag=f"{b}sb", name=f"{b}sb")
                   for b in B}
            fwh = pers.tile([P, DT], F16, tag="fwh")
            rs = pers.tile([1, NS], F32, tag="rs")    # softmax denom (debug)

            # ---- DRAM scratch: collective buffers ----
            kv_d = [dram.tile([VSZ], F16, name=f"kv_d{c}") for c in range(2)]
            x_d = dram.tile([XN], F16, name="x_d")
            kvag_x = dram.tile([NCORES * XN], F16, name="kvag_x",
                               addr_space="Shared")
            kvag_v = [dram.tile([NCORES * VSZ], F16, name=f"kvag_v{c}",
                                addr_space="Shared")
                      for c in range(2)]

            # x^T AllGather as early as possible (collectives can't read IO
            # tensors; a direct DRAM->DRAM bounce is slow, so ship the two
            # SBUF-staged x^T halves — the gather triggers at ~t=30us)

            # ---- constants (on the GpSimd DMA queue, off the load path) ----
            for b in B:
                nc.gpsimd.dma_start(bsb[b][:],
                                    B[b].rearrange("(t p) -> p t", p=P))
            fwf = pers.tile([P, DT], F32, tag="fwf")
            nc.gpsimd.dma_start(fwf[:], fw.rearrange("(t p) -> p t", p=P))
            nc.vector.tensor_copy(fwh[:], fwf[:])

            # ---- kvloc: local x^T halves + V shard, live through the
            # local-block attention; early: wv/g weights ----
            kvloc = tc.alloc_tile_pool(name="kvloc", bufs=1)
            xsT = [kvloc.tile([P, DT, 512], F16, tag=f"xsT{h}",
                              name=f"xsT{h}") for h in range(2)]
            vs = kvloc.tile([P, KTB, D], F16, tag="vs")         # V shard
            early = tc.alloc_tile_pool(name="early", bufs=1)
            wvT = early.tile([P, DT, D], F16, tag="wvT")
            gT = early.tile([P, DT, D], F16, tag="gT")

            with tc.tile_pool(name="ppj", bufs=4, space="PSUM") as ppj:
                nc.sync.dma_start(xsT[0][:], xst[:, 0])
                nc.sync.dma_start(xsT[1][:], xst[:, 1])
                x_d_v = x_d.rearrange("(p c e n) -> p c e n", p=P, c=2, e=DT)
                for h in range(2):
                    nc.scalar.dma_start(x_d_v[:, h], xsT[h][:])
                nc.gpsimd.collective_compute(
                    "AllGather", ALU.bypass,
                    replica_groups=[list(range(NCORES))],
                    ins=[x_d.opt()], outs=[kvag_x.opt()])
                nc.sync.dma_start(
                    wvT[:], wvt.rearrange("(e p) c -> p e c", p=P))
                nc.sync.dma_start(gT[:], g.rearrange("(e p) c -> p e c", p=P))
                for w in ("w1", "w2", "w3"):
                    nc.sync.dma_start(
                        wT[w][:], WT[w].rearrange("(e p) c -> p e c", p=P))

                # V halves -> ship -> AllGather (queued behind the x gather)
                for h in range(2):
                    for kt in range(h * KTH, (h + 1) * KTH):
                        for dh in range(2):
                            ps = ppj.tile([P, 512], F32, tag="ppj")
                            for et in range(DT):
                                nc.tensor.matmul(
                                    ps[:],
                                    xsT[h][:, et,
                                           (kt - h * KTH) * P:
                                           (kt - h * KTH + 1) * P],
                                    wvT[:, et, dh * 512:(dh + 1) * 512],
                                    start=(et == 0), stop=(et == DT - 1))
                            nc.scalar.copy(
                                vs[:, kt, dh * 512:(dh + 1) * 512], ps[:])
                    nc.scalar.dma_start(
                        kv_d[h].rearrange("(p t d) -> p t d", p=P, d=D),
                        vs[:, h * KTH:(h + 1) * KTH, :])
                    nc.gpsimd.collective_compute(
                        "AllGather", ALU.bypass,
                        replica_groups=[list(range(NCORES))],
                        ins=[kv_d[h].opt()], outs=[kvag_v[h].opt()])

                # U^T = G^T @ xs^T + (bq Wk): fills the collective latency
                for dt in range(DT):
                    for h in range(2):
                        ps = ppj.tile([P, 512], F32, tag="ppj")
                        for et in range(DT):
                            nc.tensor.matmul(
                                ps[:],
                                gT[:, et, dt * P:(dt + 1) * P],
                                xsT[h][:, et, :],
                                start=(et == 0), stop=(et == DT - 1))
                        nc.scalar.activation(
                            qt[:, dt, h * 512:(h + 1) * 512], ps[:],
                            AF.Identity, bias=bsb["ub"][:, dt:dt + 1])

            early.release()

            if debug:
                nc.sync.dma_start(dbg["dq"].rearrange("(t p) k -> p t k", p=P),
                                  qt[:])

            # ---- attention: 2 local blocks (from SBUF, during AllGathers)
            # then 2 chunks x 7 foreign blocks via per-core dynamic offsets
            pacc = tc.alloc_tile_pool(name="pacc", bufs=1)
            attacc = pacc.tile([P, DT, NS], F32, tag="attacc")
            rs_acc = pacc.tile([P, 2, 512], F32, tag="rs_acc")
            osb = pacc.tile([1, 14], mybir.dt.uint32, tag="osb")
            nc.gpsimd.dma_start(osb[:], boff)
            with (
                tc.tile_pool(name="kv", bufs=3) as kv,
                tc.tile_pool(name="ex", bufs=8) as exp_pool,
                tc.tile_pool(name="psc", bufs=2, space="PSUM") as psc,
                tc.tile_pool(name="pat", bufs=4, space="PSUM") as pat,
            ):
                def attn_block(kt_tile, v_tile, v_ktbase, first):
                    for qp in range(2):
                        qpsl = slice(qp * 512, (qp + 1) * 512)
                        exs = []
                        for kt in range(KTH):
                            sc = psc.tile([P, 512], F32, tag="psc")
                            for dt in range(DT):
                                nc.tensor.matmul(
                                    sc[:],
                                    kt_tile[:, dt, kt * P:(kt + 1) * P],
                                    qt[:, dt, qpsl],
                                    start=(dt == 0), stop=(dt == DT - 1))
                            ex = exp_pool.tile([P, 512], F16, tag="ex",
                                               name=f"ex{kt}")
                            nc.scalar.activation(ex[:], sc[:], AF.Exp,
                                                 scale=float(SCALE))
                            # softmax denominator: per-partition partial
                            # sums on the DVE (reduced at the end)
                            if first and kt == 0:
                                nc.vector.tensor_copy(rs_acc[:, qp, :], ex[:])
                            else:
                                nc.vector.tensor_tensor(
                                    rs_acc[:, qp, :], ex[:],
                                    rs_acc[:, qp, :], ALU.add)
                            exs.append(ex)
                        # A@V: per dt, accumulate the 4 kt matmuls in one
                        # PSUM bank (free dim 512), 4 banks rotating
                        for dt in range(DT):
                            att_ps = pat.tile([P, 512], F32, tag="pat")
                            for kt in range(KTH):
                                nc.tensor.matmul(
                                    att_ps[:],
                                    v_tile[:, v_ktbase + kt,
                                           dt * P:(dt + 1) * P],
                                    exs[kt][:],
                                    start=(kt == 0),
                                    stop=(kt == KTH - 1))
                            dsl = (slice(None), dt, qpsl)
                            if first:
                                nc.vector.tensor_copy(attacc[dsl], att_ps[:])
                            else:
                                nc.vector.tensor_tensor(
                                    attacc[dsl], att_ps[:],
                                    attacc[dsl], ALU.add)

                # foreign-block offsets into registers (Sync engine issues
                # the gathered-block DMAs)
                kofs, vofs = [], []
                for j in range(7):
                    rk = nc.sync.alloc_register(f"koff{j}")
                    nc.sync.reg_load(rk, osb[0:1, j:j + 1])
                    kofs.append(nc.sync.snap(rk, donate=True, min_val=0,
                                             max_val=7 * XN))
                    rv = nc.sync.alloc_register(f"voff{j}")
                    nc.sync.reg_load(rv, osb[0:1, 7 + j:8 + j])
                    vofs.append(nc.sync.snap(rv, donate=True, min_val=0,
                                             max_val=7 * VSZ))

                # local blocks: x^T/V already in SBUF; run under AllGather0
                attn_block(xsT[0], vs, 0, True)
                attn_block(xsT[1], vs, KTH, False)

                for ch in range(2):
                    for j in range(7):
                        ktb = kv.tile([P, DT, CH], F16, tag="ktb")
                        vb = kv.tile([P, KTH, D], F16, tag="vb")
                        nc.sync.dma_start(
                            ktb[:],
                            kvag_x[ds(kofs[j], XN)].rearrange(
                                "(p c e n) -> p c e n",
                                p=P, c=2, e=DT)[:, ch],
                            bounds_check="err")
                        nc.sync.dma_start(
                            vb[:],
                            kvag_v[ch][ds(vofs[j], VSZ)].rearrange(
                                "(p t d) -> p t d", p=P, d=D),
                            bounds_check="err")
                        attn_block(ktb, vb, 0, False)
                # reduce rs_acc across partitions, replicated to all 128
                # partitions (ones [P,P] stationary), then a parallel
                # reciprocal straight into the normalize operand
                recip_b = pacc.tile([P, 2, 512], F32, tag="recip_b")
                with tc.tile_pool(name="prs", bufs=2, space="PSUM") as prs:
                    ones_f = pacc.tile([P, P], F32, tag="ones_f")
                    nc.gpsimd.memset(ones_f[:], 1.0)
                    for qp in range(2):
                        rs_ps = prs.tile([P, 512], F32, tag="prs")
                        nc.tensor.matmul(rs_ps[:], ones_f[:],
                                         rs_acc[:, qp, :])
                        nc.vector.reciprocal(recip_b[:, qp, :], rs_ps[:])
                        if debug:
                            nc.vector.tensor_copy(
                                rs[0:1, qp * 512:(qp + 1) * 512],
                                rs_ps[0:1, :])

            # ---- normalize + MLP + final (V bias folded into b1) ----
            with (
                tc.tile_pool(name="acts", bufs=2) as acts,
                tc.tile_pool(name="pml", bufs=4, space="PSUM") as pml,
            ):
                out_sb = acts.tile([1, NS], F32, tag="out_sb")
                attn_h = acts.tile([P, DT, NS], F16, tag="y")
                for h in range(2):
                    qsl = slice(h * 512, (h + 1) * 512)
                    for dt in range(DT):
                        nc.vector.tensor_tensor(
                            attn_h[:, dt, qsl], attacc[:, dt, qsl],
                            recip_b[:, h, :], ALU.mult)
                if debug:
                    nc.sync.dma_start(dbg["drs"][:], rs[:])
                    nc.sync.dma_start(
                        dbg["datt"].rearrange("(t p) q -> p t q", p=P),
                        attn_h[:])
                cur = attn_h
                for wname, bname in (("w1", "b1"), ("w2", "b2"), ("w3", "b3")):
                    nxt = acts.tile([P, DT, NS], F16, tag="y")
                    for ft in range(DT):
                        for h in range(2):
                            ps = pml.tile([P, 512], F32, tag="pml")
                            for dt in range(DT):
                                nc.tensor.matmul(
                                    ps[:],
                                    wT[wname][:, dt, ft * P:(ft + 1) * P],
                                    cur[:, dt, h * 512:(h + 1) * 512],
                                    start=(dt == 0), stop=(dt == DT - 1))
                            nc.scalar.activation(
                                nxt[:, ft, h * 512:(h + 1) * 512], ps[:],
                                AF.Relu, bias=bsb[bname][:, ft:ft + 1])
                    if debug and wname == "w1":
                        nc.sync.dma_start(
                            dbg["dy1"].rearrange("(t p) q -> p t q", p=P),
                            nxt[:])
                    cur = nxt
                for h in range(2):
                    ps = pml.tile([1, 512], F32, tag="pfin")
                    for ft in range(DT):
                        nc.tensor.matmul(
                            ps[:], fwh[:, ft:ft + 1],
                            cur[:, ft, h * 512:(h + 1) * 512],
                            start=(ft == 0), stop=(ft == DT - 1))
                    nc.vector.tensor_copy(out_sb[0:1, h * 512:(h + 1) * 512],
                                          ps[:])
                nc.sync.dma_start(out[:], out_sb[:])
            pacc.release()
            kvloc.release()

    nc.compile()
    return nc


def _get_nc():
    if "nc" not in _CACHE:
        _CACHE["nc"] = _build()
    return _CACHE["nc"]


def _prep_shared(inputs):
    """Host-side prep: fold/transform the weights once in numpy."""
    f32 = np.float32
    Wq = np.asarray(inputs["Wq"], f32)
    Wk = np.asarray(inputs["Wk"], f32)
    Wv = np.asarray(inputs["Wv"], f32)
    W1 = np.asarray(inputs["W1"], f32)
    shared = {
        "g": np.ascontiguousarray((Wq.T @ Wk).astype(np.float16)),
        "wvt": np.ascontiguousarray(Wv.T.astype(np.float16)),
        "w1t": np.ascontiguousarray(W1.T.astype(np.float16)),
        "w2t": np.ascontiguousarray(
            np.asarray(inputs["W2"], f32).T.astype(np.float16)),
        "w3t": np.ascontiguousarray(
            np.asarray(inputs["W3"], f32).T.astype(np.float16)),
        "ub": np.ascontiguousarray(np.asarray(inputs["bq"], f32) @ Wk),
        "b1": np.ascontiguousarray(
            np.asarray(inputs["b1"], f32)
            + W1 @ np.asarray(inputs["bv"], f32)),
        "b2": np.ascontiguousarray(np.asarray(inputs["b2"], f32)),
        "b3": np.ascontiguousarray(np.asarray(inputs["b3"], f32)),
        "fw": np.ascontiguousarray(
            np.asarray(inputs["final_weight"], f32).reshape(D)),
    }
    return shared


def _boff(c):
    """Element offsets of the 7 foreign (x^T, V) blocks for core c."""
    blks = [(c + 1 + j) % NCORES for j in range(7)]
    ks = [b * XN for b in blks]
    vso = [b * VSZ for b in blks]
    return np.array([ks + vso], dtype=np.uint32)


def _pack_xst(x_shard):
    """[NS, D] fp32 -> [P, 2, DT, 512] fp16, x^T packed chunk-major."""
    a = x_shard.T.astype(np.float16)           # [D, NS]
    a = a.reshape(DT, P, 2, 512)               # [e, p, ch, n']
    return np.ascontiguousarray(a.transpose(1, 2, 0, 3))


def kernel(**inputs):
    nc = _get_nc()
    x = np.asarray(inputs["x"], dtype=np.float32)
    shared = _prep_shared(inputs)
    in_maps = []
    for c in range(NCORES):
        m = dict(shared)
        m["xst"] = _pack_xst(x[c * NS:(c + 1) * NS, :])
        m["boff"] = _boff(c)
        in_maps.append(m)
    res = bass_utils.run_bass_kernel_spmd(
        nc, in_maps, core_ids=list(range(NCORES)))
    if os.environ.get("K_DEBUG"):
        kernel.debug_results = res.results
    return np.concatenate(
        [res.results[c]["out"].reshape(NS) for c in range(NCORES)])
